# revision 1
# baseline (speedup 1.0000x reference)
"""Trainium2 Bass kernel for nn_MultiHeadAttention (B=1, S=4096, E=768, H=12, D=64).

Causal multi-head attention, sequence-parallel across 8 NeuronCores.

Strategy (single SPMD program, per-core variation is data-only):
- Query rows are split into 32 global chunks of 128 rows. Core c owns chunks
  G(g) = 8g + c for g = 0..3. Chunk g runs a fixed slot loop of 8(g+1)
  k-blocks (uniform across cores); causally-dead slots are killed by a
  per-slot exp bias of -60 (data), and the diagonal block is handled in the
  last slot with separately-projected "own" K/V tiles plus a triangular mask.
- Scores are computed transposed (S^T[k, q], k on partitions) so the exp
  output P^T feeds the attn@V matmul directly. Row sums l come from a fused
  ones-column appended to V. 1/l = exp(-ln(l)) on the scalar engine (both
  functions live in one activation table set). The 1/l broadcast across
  partitions is a K=1 matmul against a ones vector.
- All matmuls in bf16 (fp32 PSUM accumulation); x is transposed via DMA
  xbar transpose after a bf16 cast.
- Every core computes the full K/V projection locally (replicated), its own
  Q, and its own 512 output rows including the final out-projection + bias.
"""

import os
from contextlib import ExitStack

import numpy as np
import ml_dtypes

import concourse.bass as bass
import concourse.tile as tile
from concourse import bacc, bass_utils, mybir

F32 = mybir.dt.float32
F32R = mybir.dt.float32r
BF16 = mybir.dt.bfloat16

N_CORES = 8
S, E, H, D = 4096, 768, 12, 64
P = 128
NCH = 4  # chunks per core
SLOTS = [8, 16, 24, 32]  # slot count for chunk g
SLOT_BASE = [0, 8, 24, 48]  # cumulative
TOT_SLOTS = 80
EC = E // P  # 6 e-chunks of 128
NPAIR = 6  # head pairs
QOWN = NCH * P  # 512 own q rows
NEG = -60.0  # exp bias for masked slots: exp(-60 +- 4) == 0 numerically


def build_program():
    nc = bacc.Bacc("TRN2", target_bir_lowering=False, debug=False, num_devices=N_CORES)

    x = nc.dram_tensor("x", [S, E], F32, kind="ExternalInput").ap()
    xq = nc.dram_tensor("xq", [QOWN, E], F32, kind="ExternalInput").ap()
    wq = nc.dram_tensor("wq", [E, E], F32, kind="ExternalInput").ap()
    wk = nc.dram_tensor("wk", [E, E], F32, kind="ExternalInput").ap()
    wv = nc.dram_tensor("wv", [E, E], F32, kind="ExternalInput").ap()
    wo = nc.dram_tensor("wo", [D, H, E], F32, kind="ExternalInput").ap()
    bob = nc.dram_tensor("bob", [P, E], F32, kind="ExternalInput").ap()
    btab = nc.dram_tensor("btab", [P, TOT_SLOTS], F32, kind="ExternalInput").ap()
    dmask = nc.dram_tensor("dmask", [P, NPAIR * P], BF16, kind="ExternalInput").ap()
    y = nc.dram_tensor("y", [QOWN, E], F32, kind="ExternalOutput").ap()

    with tile.TileContext(nc) as tc, ExitStack() as top:
        const = top.enter_context(tc.tile_pool(name="const", bufs=1))
        big = top.enter_context(tc.tile_pool(name="big", bufs=1))
        own = top.enter_context(tc.tile_pool(name="own", bufs=1))
        sc_p = top.enter_context(tc.tile_pool(name="scp", bufs=2, space="PSUM"))
        ctx_p = top.enter_context(tc.tile_pool(name="ctxp", bufs=2, space="PSUM"))

        # ---- constants ----
        btab_sb = const.tile([P, TOT_SLOTS], F32, tag="btab")
        nc.sync.dma_start(out=btab_sb, in_=btab)
        dmask_bf = const.tile([P, NPAIR * P], BF16, tag="dmaskb")
        nc.sync.dma_start(out=dmask_bf, in_=dmask)
        bob_sb = const.tile([P, E], F32, tag="bob")
        nc.sync.dma_start(out=bob_sb, in_=bob)
        ones_f = const.tile([P, D], F32, tag="onesf")
        nc.vector.memset(ones_f, 1.0)
        ones_sb = const.tile([P, D], F32R, tag="ones")
        nc.scalar.copy(out=ones_sb, in_=ones_f)
        zb = const.tile([P, 512], BF16, tag="zb")
        nc.vector.memset(zb, 0.0)

        # ---- persistent bf16 operands ----
        kt = big.tile([P, NPAIR, S], BF16, tag="kt")  # K^T, head pairs on partitions
        vt = big.tile([P, S // P, H * (D + 1)], BF16, tag="vt")  # V + ones cols
        # own Q^T, zero-padded variant pairs: qtp[:, pc, 0, :] has head-pair
        # rows 64:128 zeroed (selects the even head), qtp[:, pc, 1, :] has rows
        # 0:64 zeroed (odd head). Scores contract over the full 128 partitions
        # (the dead half contributes 0), keeping every matmul operand at base
        # partition 0 (base-64 operands hang this HW path), and both heads of
        # a pair ride one N=256 matmul with the same stationary K tile.
        qtp = own.tile([P, NPAIR, 2, QOWN], BF16, tag="qtp")
        nc.vector.memset(qtp[D:P, :, 0, :], 0.0)
        nc.vector.memset(qtp[0:D, :, 1, :], 0.0)
        kto = own.tile([P, NPAIR, QOWN], BF16, tag="kto")  # own K^T (diagonal)
        vto = own.tile([P, NCH, H * (D + 1)], BF16, tag="vto")  # own V (diagonal)

        vt_v = vt.rearrange("p b (h c) -> p b h c", c=D + 1)
        vto_v = vto.rearrange("p b (h c) -> p b h c", c=D + 1)

        # ================= projection phase (nested pools) =================
        with ExitStack() as proj:
            wpool = proj.enter_context(tc.tile_pool(name="wpool", bufs=1))
            wstage = proj.enter_context(tc.tile_pool(name="wstage", bufs=2))
            xst_p = proj.enter_context(tc.tile_pool(name="xst", bufs=2))
            xbf_p = proj.enter_context(tc.tile_pool(name="xbf", bufs=2))
            xt_p = proj.enter_context(tc.tile_pool(name="xt", bufs=2))
            xq_pool = proj.enter_context(tc.tile_pool(name="xqp", bufs=1))

            wqb = wpool.tile([P, EC, E], BF16, tag="wqb")
            wkb = wpool.tile([P, EC, E], BF16, tag="wkb")
            wvb = wpool.tile([P, EC, E], BF16, tag="wvb")
            for w_dram, w_bf in ((wq, wqb), (wk, wkb), (wv, wvb)):
                for h3 in range(3):  # stream in thirds
                    wst = wstage.tile([P, 2, E], F32, tag="wst")
                    nc.sync.dma_start(
                        out=wst,
                        in_=w_dram.rearrange("(c p) n -> p c n", p=P)[:, 2 * h3 : 2 * h3 + 2, :],
                    )
                    nc.vector.tensor_copy(out=w_bf[:, 2 * h3 : 2 * h3 + 2, :], in_=wst)

            xqt = xq_pool.tile([P, EC, QOWN], BF16, tag="xqt")  # own x^T

            def load_transposed(src, sblk0, nblk, dst, dst_col0):
                """DMA nblk [128,768] row-blocks, cast bf16, xbar-transpose into
                dst[:, ec, dst_col0 : dst_col0 + nblk*128]."""
                xs = xst_p.tile([P, nblk, E], F32, tag="xst")
                nc.sync.dma_start(
                    out=xs,
                    in_=src.rearrange("(s p) e -> p s e", p=P)[:, sblk0 : sblk0 + nblk, :],
                )
                xb = xbf_p.tile([P, nblk, E], BF16, tag="xbf")
                nc.vector.tensor_copy(out=xb, in_=xs)
                for b in range(nblk):
                    for ec in range(EC):
                        nc.sync.dma_start_transpose(
                            out=dst[:, ec, dst_col0 + b * P : dst_col0 + (b + 1) * P],
                            in_=xb[:, b, ec * P : (ec + 1) * P],
                        )

            for gi in range(2):
                load_transposed(xq, 2 * gi, 2, xqt, gi * 256)

            def project_pairs(w_bf, dst, src_t, ncols, split=False):
                for pc in range(NPAIR):
                    ps = sc_p.tile([P, ncols], F32, tag="sc", name=f"ps_p{pc}")
                    for ec in range(EC):
                        nc.tensor.matmul(
                            out=ps,
                            lhsT=w_bf[:, ec, pc * P : (pc + 1) * P],
                            rhs=src_t[:, ec, :],
                            start=(ec == 0),
                            stop=(ec == EC - 1),
                        )
                    if split:
                        nc.vector.tensor_copy(out=dst[0:D, pc, 0, :], in_=ps[0:D, :])
                        nc.vector.tensor_copy(out=dst[D:P, pc, 1, :], in_=ps[D:P, :])
                    else:
                        nc.scalar.copy(out=dst[:, pc, :], in_=ps)

            project_pairs(wqb, qtp, xqt, QOWN, split=True)
            project_pairs(wkb, kto, xqt, QOWN)

            def project_v(dst_view, xt_tile, xt_col0):
                for half in range(2):
                    ps = sc_p.tile([P, E // 2], F32, tag="sc", name=f"ps_v{half}")
                    for ec in range(EC):
                        nc.tensor.matmul(
                            out=ps,
                            lhsT=xt_tile[:, ec, xt_col0 : xt_col0 + P],
                            rhs=wvb[:, ec, half * 384 : (half + 1) * 384],
                            start=(ec == 0),
                            stop=(ec == EC - 1),
                        )
                    nc.vector.tensor_copy(
                        out=dst_view[:, half * 6 : (half + 1) * 6, 0:D],
                        in_=ps.rearrange("p (h d) -> p h d", d=D),
                    )
                nc.vector.memset(dst_view[:, :, D : D + 1], 1.0)

            for qb in range(NCH):
                project_v(vto_v[:, qb, :, :], xqt, qb * P)

            # ---- full K^T and V from x (the replicated part) ----
            for sg in range(S // 512):
                xt_g = xt_p.tile([P, EC, 512], BF16, tag="xtg", name=f"xt_{sg}")
                for gi in range(2):
                    load_transposed(x, 4 * sg + 2 * gi, 2, xt_g, gi * 256)
                for pc in range(NPAIR):
                    ps = sc_p.tile([P, 512], F32, tag="sc", name=f"ps_k{pc}")
                    for ec in range(EC):
                        nc.tensor.matmul(
                            out=ps,
                            lhsT=wkb[:, ec, pc * P : (pc + 1) * P],
                            rhs=xt_g[:, ec, :],
                            start=(ec == 0),
                            stop=(ec == EC - 1),
                        )
                    nc.scalar.copy(out=kt[:, pc, sg * 512 : (sg + 1) * 512], in_=ps)
                for b in range(4):
                    project_v(vt_v[:, 4 * sg + b, :, :], xt_g, b * P)

        phase = os.environ.get("KERNEL_PHASE", "full")
        if phase == "proj":
            # debug: dump projections into y and stop
            dbg = own.tile([P, E], F32, tag="dbg")
            nc.scalar.copy(out=dbg[:, 0:E], in_=kt[:, 0, 0:E])
            nc.sync.dma_start(out=y[0:P, :], in_=dbg)
            dbg2 = own.tile([P, E], F32, tag="dbg2")
            nc.scalar.copy(out=dbg2, in_=vt[:, 0, 0:E])
            nc.sync.dma_start(out=y[P : 2 * P, :], in_=dbg2)
            dbg3 = own.tile([P, E], F32, tag="dbg3")
            nc.scalar.copy(out=dbg3[:, 0:QOWN], in_=qt[:, 0, :])
            nc.scalar.copy(out=dbg3[:, QOWN:E], in_=kto[:, 1, 0 : E - QOWN])
            nc.sync.dma_start(out=y[2 * P : 3 * P, :], in_=dbg3)
            dbg4 = own.tile([P, E], F32, tag="dbg4")
            nc.scalar.copy(out=dbg4, in_=vto[:, 0, 0:E])
            nc.sync.dma_start(out=y[3 * P : 4 * P, :], in_=dbg4)

        n_chunks_run = 0 if phase == "proj" else NCH
        if phase.startswith("att"):
            n_chunks_run = int(phase[3:])
        skip_epi = bool(int(os.environ.get("KERNEL_SKIP_EPI", "0")))

        # ================= attention phase =================
        with ExitStack() as att:
            wo_pool = att.enter_context(tc.tile_pool(name="wop", bufs=1))
            wob = wo_pool.tile([D, H, E], BF16, tag="wob")
            with ExitStack() as wos:
                wo_st = wos.enter_context(tc.tile_pool(name="wost", bufs=1))
                for h2 in range(2):
                    wost = wo_st.tile([D, EC, E], F32, tag="wost", name=f"wo_{h2}")
                    nc.sync.dma_start(out=wost, in_=wo[:, 6 * h2 : 6 * h2 + 6, :])
                    nc.vector.tensor_copy(out=wob[:, 6 * h2 : 6 * h2 + 6, :], in_=wost)

            pt_p = att.enter_context(tc.tile_pool(name="pt", bufs=1))
            misc = att.enter_context(tc.tile_pool(name="misc", bufs=1))

            for g in range(n_chunks_run):
                nslot = SLOTS[g]
                ctx_t = [
                    ctx_p.tile([D + 1, NPAIR * P], F32, tag="ctx", name=f"ctx_g{g}_{i}")
                    for i in range(2)
                ]

                def ctx_fence(start):
                    # bank-wide zero matmuls fencing the per-head accumulation:
                    # one start=True / stop=True group per PSUM bank, with all
                    # real ctx matmuls as flags=0 accumulates in between. The
                    # full-bank writes give WAW deps ordering them correctly.
                    for t in ctx_t:
                        for lo, n in ((0, 512), (512, 256)):
                            nc.tensor.matmul(
                                out=t[0 : D + 1, lo : lo + n],
                                lhsT=zb[0:1, 0 : D + 1],
                                rhs=zb[0:1, 0:n],
                                start=start,
                                stop=not start,
                            )

                ctx_fence(start=True)
                for s in range(nslot):
                    is_diag = s == nslot - 1
                    for hg in range(2):
                        sc = sc_p.tile(
                            [P, NPAIR * P], F32, tag="sc", name=f"sc_{g}_{s}_{hg}"
                        )
                        for pl in range(3):
                            pc = hg * 3 + pl
                            if is_diag:
                                lhsT = kto[:, pc, g * P : (g + 1) * P]
                            else:
                                lhsT = kt[:, pc, s * P : (s + 1) * P]
                            # single-shot scores; interleaved start=True groups in
                            # one bank are HW-safe (per-element data unaffected).
                            # rhs [128, 2, 128] = both zero-padded Q variants ->
                            # out [128, 256] = both heads of the pair.
                            nc.tensor.matmul(
                                out=sc[:, pl * 2 * P : (pl + 1) * 2 * P],
                                lhsT=lhsT,
                                rhs=qtp[:, pc, :, g * P : (g + 1) * P],
                                start=True,
                                stop=True,
                                skip_group_check=True,
                            )
                        pt = pt_p.tile(
                            [P, NPAIR * P], BF16, tag="pt", bufs=2, name=f"pt_{g}_{s}_{hg}"
                        )
                        sg_idx = SLOT_BASE[g] + s
                        nc.scalar.activation(
                            out=pt,
                            in_=sc,
                            func=mybir.ActivationFunctionType.Exp,
                            bias=btab_sb[:, sg_idx : sg_idx + 1],
                            scale=0.125,
                        )
                        if is_diag:
                            ptm = pt_p.tile(
                                [P, NPAIR * P], BF16, tag="ptm", name=f"ptm_{g}_{hg}"
                            )
                            nc.vector.tensor_mul(out=ptm, in0=pt, in1=dmask_bf)
                            pt = ptm
                        for hl in range(6):
                            h = hg * 6 + hl
                            vsrc = vto_v[:, g, h, :] if is_diag else vt_v[:, s, h, :]
                            nc.tensor.matmul(
                                out=ctx_t[hg][:, hl * P : (hl + 1) * P],
                                lhsT=vsrc,
                                rhs=pt[:, hl * P : (hl + 1) * P],
                                start=False,
                                stop=False,
                            )
                ctx_fence(start=False)
                if skip_epi:
                    dbg5 = misc.tile([P, E], F32, tag="dbg5", name=f"dbg5_{g}")
                    nc.scalar.copy(out=dbg5[0 : D + 1, :], in_=ctx_t[0][:, :])
                    nc.sync.dma_start(out=y[g * P : (g + 1) * P, :], in_=dbg5)
                    continue
                # ---- epilogue: 1/l, normalize, out-projection ----
                ctxn = []
                for hg in range(2):
                    # l lives on PSUM partition 64 (the V ones-column row); PE
                    # and ACT only work from base partition 0 here, so stage it
                    # to partition 0: DVE copy (partition-locked) + tiny
                    # SBUF->SBUF DMA partition move.
                    lrow = misc.tile([P, NPAIR * P], F32, tag="lrow", name=f"lr{g}{hg}")
                    nc.vector.tensor_copy(
                        out=lrow[D : D + 1, :], in_=ctx_t[hg][D : D + 1, :]
                    )
                    nc.sync.dma_start(out=lrow[0:1, :], in_=lrow[D : D + 1, :])
                    lln = misc.tile([P, NPAIR * P], F32, tag="lln", name=f"lln{g}{hg}")
                    nc.scalar.activation(
                        out=lln[0:1, :],
                        in_=lrow[0:1, :],
                        func=mybir.ActivationFunctionType.Ln,
                    )
                    linv = misc.tile([P, NPAIR * P], F32R, tag="linv", name=f"li{g}{hg}")
                    nc.scalar.activation(
                        out=linv[0:1, :],
                        in_=lln[0:1, :],
                        func=mybir.ActivationFunctionType.Exp,
                        scale=-1.0,
                    )
                    bc = sc_p.tile([D, NPAIR * P], F32, tag="sc", name=f"bc{g}{hg}")
                    for lo, n in ((0, 512), (512, 256)):  # bank-aligned pieces
                        nc.tensor.matmul(
                            out=bc[:, lo : lo + n],
                            lhsT=ones_sb[0:1, 0:D],
                            rhs=linv[0:1, lo : lo + n],
                            start=True,
                            stop=True,
                        )
                    bcs = misc.tile(
                        [D, NPAIR * P], F32, tag="bcs", bufs=2, name=f"bcs{g}{hg}"
                    )
                    nc.scalar.copy(out=bcs, in_=bc)
                    cn = misc.tile(
                        [D, NPAIR * P], BF16, tag="ctxn", bufs=2, name=f"cn{g}{hg}"
                    )
                    nc.vector.tensor_mul(out=cn, in0=ctx_t[hg][0:D, :], in1=bcs)
                    ctxn.append(cn)
                for fh in range(2):
                    op = sc_p.tile([P, 384], F32, tag="sc", name=f"op{g}{fh}")
                    for h in range(H):
                        nc.tensor.matmul(
                            out=op,
                            lhsT=ctxn[h // 6][:, (h % 6) * P : (h % 6 + 1) * P],
                            rhs=wob[:, h, fh * 384 : (fh + 1) * 384],
                            start=(h == 0),
                            stop=(h == H - 1),
                        )
                    outs = misc.tile([P, 384], F32, tag="outs", bufs=2, name=f"ou{g}{fh}")
                    nc.vector.tensor_add(
                        out=outs, in0=op, in1=bob_sb[:, fh * 384 : (fh + 1) * 384]
                    )
                    nc.sync.dma_start(
                        out=y[g * P : (g + 1) * P, fh * 384 : (fh + 1) * 384], in_=outs
                    )

    nc.compile()
    return nc


_NC_CACHE = None


def _get_program():
    global _NC_CACHE
    if _NC_CACHE is None:
        _NC_CACHE = build_program()
    return _NC_CACHE


def _host_inputs(x, Wq, Wk, Wv, Wo, bo):
    """Build per-core input maps."""
    x = np.ascontiguousarray(x.reshape(S, E), dtype=np.float32)
    wo_arr = np.ascontiguousarray(
        Wo.reshape(H, D, E).transpose(1, 0, 2), dtype=np.float32
    )
    bob = np.ascontiguousarray(np.broadcast_to(bo.astype(np.float32), (P, E)))
    # diagonal mask, replicated per head-group lane: [k, hl*128 + q] = k <= q
    tri = (np.arange(P)[:, None] <= np.arange(P)[None, :]).astype(np.float32)
    dmask = np.ascontiguousarray(np.tile(tri, (1, NPAIR)).astype(ml_dtypes.bfloat16))

    in_maps = []
    for c in range(N_CORES):
        chunks = [8 * g + c for g in range(NCH)]
        xq = np.concatenate([x[gc * P : (gc + 1) * P] for gc in chunks], axis=0)
        btab = np.zeros((P, TOT_SLOTS), dtype=np.float32)
        for g in range(NCH):
            diagk = chunks[g]
            for s in range(SLOTS[g]):
                if s == SLOTS[g] - 1 or s < diagk:
                    v = 0.0  # diagonal slot or fully-valid block
                else:
                    v = NEG  # causally dead block
                btab[:, SLOT_BASE[g] + s] = v
        in_maps.append(
            {
                "x": x,
                "xq": np.ascontiguousarray(xq),
                "wq": np.ascontiguousarray(Wq, dtype=np.float32),
                "wk": np.ascontiguousarray(Wk, dtype=np.float32),
                "wv": np.ascontiguousarray(Wv, dtype=np.float32),
                "wo": wo_arr,
                "bob": bob,
                "btab": btab,
                "dmask": dmask,
            }
        )
    return in_maps


def kernel(x, Wq, Wk, Wv, Wo, bo, mask=None, **_ignored):
    nc = _get_program()
    in_maps = _host_inputs(
        np.asarray(x), np.asarray(Wq), np.asarray(Wk), np.asarray(Wv),
        np.asarray(Wo), np.asarray(bo),
    )
    trace = bool(int(os.environ.get("BASS_KERNEL_TRACE", "0")))
    res = bass_utils.run_bass_kernel_spmd(
        nc, in_maps, core_ids=list(range(N_CORES)), trace=trace
    )
    if trace:
        kernel.last_results = res
    out = np.empty((S, E), dtype=np.float32)
    for c in range(N_CORES):
        yc = res.results[c]["y"]
        for g in range(NCH):
            gc = 8 * g + c
            out[gc * P : (gc + 1) * P] = yc[g * P : (g + 1) * P]
    return out.reshape(1, S, E)



# revision 5
# speedup vs baseline: 2.0090x; 2.0090x over previous
"""Trainium2 Bass kernel for nn_MultiHeadAttention (B=1, S=4096, E=768, H=12, D=64).

Causal multi-head attention, sequence-parallel across 8 NeuronCores.

Strategy (single SPMD program, per-core variation is data-only):
- Query rows are split into 32 global chunks of 128 rows. Core c owns chunks
  G(g) = 8g + c for g = 0..3. Chunk g runs a fixed slot loop of 8(g+1)
  k-blocks (uniform across cores); causally-dead slots are killed by a
  per-slot exp bias of -60 (data), and the diagonal block is handled in the
  last slot with separately-projected "own" K/V tiles plus a triangular mask.
- Scores are computed transposed (S^T[k, q], k on partitions) so the exp
  output P^T feeds the attn@V matmul directly. Row sums l come from a fused
  ones-column PREPENDED to V (so l lands on PSUM partition 0). The epilogue
  is entirely ACT-free: DVE copies ctx out of PSUM, reciprocal_approx_fast
  gives 1/l, a K=1 matmul broadcasts it across partitions, DVE multiplies.
  The scalar engine therefore runs nothing but the exp stream (one table
  set, loaded once).
- All matmuls in bf16 (fp32 PSUM accumulation). x arrives HOST-transposed
  and bf16-cast (as x^T tiles), weights arrive bf16 pre-arranged, so the
  projection phase is pure dense matmuls with double-buffered DMA.
"""

import os
from contextlib import ExitStack

import numpy as np
import ml_dtypes

import concourse.bass as bass
import concourse.tile as tile
from concourse import bacc, bass_utils, mybir

F32 = mybir.dt.float32
BF16 = mybir.dt.bfloat16

N_CORES = 8
S, E, H, D = 4096, 768, 12, 64
P = 128
NCH = 4  # chunks per core
SLOTS = [8, 16, 24, 32]  # slot count for chunk g
SLOT_BASE = [0, 8, 24, 48]  # cumulative
TOT_SLOTS = 80
EC = E // P  # 6 e-chunks of 128
NPAIR = 6  # head pairs
QOWN = NCH * P  # 512 own q rows
NSG = S // 512  # 8 column-groups of x^T
NEG = -60.0  # exp bias for masked slots: exp(-60 +- 4) == 0 numerically
DV = D + 1  # v row block: ones col + 64 dims


def build_program():
    nc = bacc.Bacc("TRN2", target_bir_lowering=False, debug=False, num_devices=N_CORES)

    # host-prepped inputs (bf16, pre-transposed / pre-arranged)
    xt = nc.dram_tensor("xt", [P, NSG, EC, 512], BF16, kind="ExternalInput").ap()
    xqt = nc.dram_tensor("xqt", [P, EC, QOWN], BF16, kind="ExternalInput").ap()
    wq = nc.dram_tensor("wq", [P, EC, E], BF16, kind="ExternalInput").ap()
    wk = nc.dram_tensor("wk", [P, EC, E], BF16, kind="ExternalInput").ap()
    wv = nc.dram_tensor("wv", [P, EC, E], BF16, kind="ExternalInput").ap()
    wo = nc.dram_tensor("wo", [D, H, E], BF16, kind="ExternalInput").ap()
    bob = nc.dram_tensor("bob", [P, E], F32, kind="ExternalInput").ap()
    btab = nc.dram_tensor("btab", [P, TOT_SLOTS], F32, kind="ExternalInput").ap()
    dmask = nc.dram_tensor("dmask", [P, NPAIR * P], BF16, kind="ExternalInput").ap()
    y = nc.dram_tensor("y", [QOWN, E], F32, kind="ExternalOutput").ap()

    with tile.TileContext(nc) as tc, ExitStack() as top:
        const = top.enter_context(tc.tile_pool(name="const", bufs=1))
        big = top.enter_context(tc.tile_pool(name="big", bufs=1))
        own = top.enter_context(tc.tile_pool(name="own", bufs=1))
        sc_p = top.enter_context(tc.tile_pool(name="scp", bufs=2, space="PSUM"))
        ctx_p = top.enter_context(tc.tile_pool(name="ctxp", bufs=2, space="PSUM"))

        # ---- constants ----
        btab_sb = const.tile([P, TOT_SLOTS], F32, tag="btab")
        nc.sync.dma_start(out=btab_sb, in_=btab)
        dmask_bf = const.tile([P, NPAIR * P], BF16, tag="dmaskb")
        nc.sync.dma_start(out=dmask_bf, in_=dmask)
        bob_sb = const.tile([P, E], F32, tag="bob")
        nc.sync.dma_start(out=bob_sb, in_=bob)
        ones_sb = const.tile([P, D], F32, tag="ones")
        nc.vector.memset(ones_sb, 1.0)
        zb = const.tile([P, 512], BF16, tag="zb")
        nc.vector.memset(zb, 0.0)

        # ---- persistent bf16 operands ----
        kt = big.tile([P, NPAIR, S], BF16, tag="kt")  # K^T, head pairs on partitions
        vt = big.tile([P, S // P, H * DV], BF16, tag="vt")  # ones col + V
        # own Q^T, zero-padded variant pairs: qtp[:, pc, 0, :] has head-pair
        # rows 64:128 zeroed (selects the even head), qtp[:, pc, 1, :] has rows
        # 0:64 zeroed (odd head). Scores contract over the full 128 partitions
        # (the dead half contributes 0), keeping every matmul operand at base
        # partition 0, and both heads of a pair ride one N=256 matmul with the
        # same stationary K tile.
        qtp = own.tile([P, NPAIR, 2, QOWN], BF16, tag="qtp")
        nc.vector.memset(qtp[D:P, :, 0, :], 0.0)
        nc.vector.memset(qtp[0:D, :, 1, :], 0.0)
        kto = own.tile([P, NPAIR, QOWN], BF16, tag="kto")  # own K^T (diagonal)
        vto = own.tile([P, NCH, H * DV], BF16, tag="vto")  # own ones+V (diagonal)

        vt_v = vt.rearrange("p b (h c) -> p b h c", c=DV)
        vto_v = vto.rearrange("p b (h c) -> p b h c", c=DV)
        nc.vector.memset(vt_v[:, :, :, D : D + 1], 1.0)
        nc.vector.memset(vto_v[:, :, :, D : D + 1], 1.0)

        # ================= projection phase =================
        with ExitStack() as proj:
            wpool = proj.enter_context(tc.tile_pool(name="wpool", bufs=1))
            xq_pool = proj.enter_context(tc.tile_pool(name="xqp", bufs=1))
            xt_p = proj.enter_context(tc.tile_pool(name="xtp", bufs=2))

            wqb = wpool.tile([P, EC, E], BF16, tag="wqb")
            wkb = wpool.tile([P, EC, E], BF16, tag="wkb")
            wvb = wpool.tile([P, EC, E], BF16, tag="wvb")
            nc.sync.dma_start(out=wqb, in_=wq)
            nc.sync.dma_start(out=wkb, in_=wk)
            nc.sync.dma_start(out=wvb, in_=wv)

            xqt_sb = xq_pool.tile([P, EC, QOWN], BF16, tag="xqt")
            nc.sync.dma_start(out=xqt_sb, in_=xqt)

            def project_pairs(w_bf, dst, src_t, ncols, split=False):
                for pc in range(NPAIR):
                    ps = sc_p.tile([P, ncols], F32, tag="sc", name=f"ps_p{pc}")
                    for ec in range(EC):
                        nc.tensor.matmul(
                            out=ps,
                            lhsT=w_bf[:, ec, pc * P : (pc + 1) * P],
                            rhs=src_t[:, ec, :],
                            start=(ec == 0),
                            stop=(ec == EC - 1),
                        )
                    if split:
                        nc.vector.tensor_copy(out=dst[0:D, pc, 0, :], in_=ps[0:D, :])
                        nc.vector.tensor_copy(out=dst[D:P, pc, 1, :], in_=ps[D:P, :])
                    else:
                        nc.scalar.copy(out=dst[:, pc, :], in_=ps)

            project_pairs(wqb, qtp, xqt_sb, QOWN, split=True)
            project_pairs(wkb, kto, xqt_sb, QOWN)

            def project_v(dst_view, xt_tile, xt_col0):
                for half in range(2):
                    ps = sc_p.tile([P, E // 2], F32, tag="sc", name=f"ps_v{half}")
                    for ec in range(EC):
                        nc.tensor.matmul(
                            out=ps,
                            lhsT=xt_tile[:, ec, xt_col0 : xt_col0 + P],
                            rhs=wvb[:, ec, half * 384 : (half + 1) * 384],
                            start=(ec == 0),
                            stop=(ec == EC - 1),
                        )
                    nc.vector.tensor_copy(
                        out=dst_view[:, half * 6 : (half + 1) * 6, 0:D],
                        in_=ps.rearrange("p (h d) -> p h d", d=D),
                    )

            for qb in range(NCH):
                project_v(vto_v[:, qb, :, :], xqt_sb, qb * P)

            # ---- full K^T and V from x^T (the replicated part) ----
            for sg in range(NSG):
                xt_g = xt_p.tile([P, EC, 512], BF16, tag="xtg", name=f"xt_{sg}")
                nc.sync.dma_start(out=xt_g, in_=xt[:, sg, :, :])
                for pc in range(NPAIR):
                    ps = sc_p.tile([P, 512], F32, tag="sc", name=f"ps_k{pc}")
                    for ec in range(EC):
                        nc.tensor.matmul(
                            out=ps,
                            lhsT=wkb[:, ec, pc * P : (pc + 1) * P],
                            rhs=xt_g[:, ec, :],
                            start=(ec == 0),
                            stop=(ec == EC - 1),
                        )
                    nc.scalar.copy(out=kt[:, pc, sg * 512 : (sg + 1) * 512], in_=ps)
                for b in range(4):
                    project_v(vt_v[:, 4 * sg + b, :, :], xt_g, b * P)

        phase = os.environ.get("KERNEL_PHASE", "full")
        n_chunks_run = 0 if phase == "proj" else NCH
        if phase.startswith("att"):
            n_chunks_run = int(phase[3:])
        skip_epi = bool(int(os.environ.get("KERNEL_SKIP_EPI", "0")))

        # ================= attention phase =================
        with ExitStack() as att:
            wo_pool = att.enter_context(tc.tile_pool(name="wop", bufs=1))
            wob = wo_pool.tile([D, H, E], BF16, tag="wob")
            nc.sync.dma_start(out=wob, in_=wo)

            pt_p = att.enter_context(tc.tile_pool(name="pt", bufs=1))
            misc = att.enter_context(tc.tile_pool(name="misc", bufs=1))

            for g in range(n_chunks_run):
                nslot = SLOTS[g]
                ctx_t = [
                    ctx_p.tile([DV, NPAIR * P], F32, tag="ctx", name=f"ctx_g{g}_{i}")
                    for i in range(2)
                ]

                def ctx_fence(start):
                    # bank-wide zero matmuls fencing the per-head accumulation:
                    # one start=True / stop=True group per PSUM bank, with all
                    # real ctx matmuls as flags=0 accumulates in between. The
                    # full-bank writes give WAW deps ordering them correctly.
                    for t in ctx_t:
                        for lo, n in ((0, 512), (512, 256)):
                            nc.tensor.matmul(
                                out=t[0:DV, lo : lo + n],
                                lhsT=zb[0:1, 0:DV],
                                rhs=zb[0:1, 0:n],
                                start=start,
                                stop=not start,
                            )

                ctx_fence(start=True)
                for s in range(nslot):
                    is_diag = s == nslot - 1
                    for hg in range(2):
                        sc = sc_p.tile(
                            [P, NPAIR * P], F32, tag="sc", name=f"sc_{g}_{s}_{hg}"
                        )
                        for pl in range(3):
                            pc = hg * 3 + pl
                            if is_diag:
                                lhsT = kto[:, pc, g * P : (g + 1) * P]
                            else:
                                lhsT = kt[:, pc, s * P : (s + 1) * P]
                            # single-shot scores; interleaved start=True groups in
                            # one bank are HW-safe (per-element data unaffected).
                            # rhs [128, 2, 128] = both zero-padded Q variants ->
                            # out [128, 256] = both heads of the pair.
                            nc.tensor.matmul(
                                out=sc[:, pl * 2 * P : (pl + 1) * 2 * P],
                                lhsT=lhsT,
                                rhs=qtp[:, pc, :, g * P : (g + 1) * P],
                                start=True,
                                stop=True,
                                skip_group_check=True,
                            )
                        pt = pt_p.tile(
                            [P, NPAIR * P], BF16, tag="pt", bufs=2, name=f"pt_{g}_{s}_{hg}"
                        )
                        sg_idx = SLOT_BASE[g] + s
                        nc.scalar.activation(
                            out=pt,
                            in_=sc,
                            func=mybir.ActivationFunctionType.Exp,
                            bias=btab_sb[:, sg_idx : sg_idx + 1],
                            scale=0.125,
                        )
                        if is_diag:
                            ptm = pt_p.tile(
                                [P, NPAIR * P], BF16, tag="ptm", name=f"ptm_{g}_{hg}"
                            )
                            nc.vector.tensor_mul(out=ptm, in0=pt, in1=dmask_bf)
                            pt = ptm
                        for hl in range(6):
                            h = hg * 6 + hl
                            vsrc = vto_v[:, g, h, :] if is_diag else vt_v[:, s, h, :]
                            nc.tensor.matmul(
                                out=ctx_t[hg][:, hl * P : (hl + 1) * P],
                                lhsT=vsrc,
                                rhs=pt[:, hl * P : (hl + 1) * P],
                                start=False,
                                stop=False,
                            )
                ctx_fence(start=False)
                if skip_epi:
                    dbg5 = misc.tile([P, E], F32, tag="dbg5", name=f"dbg5_{g}")
                    nc.scalar.copy(out=dbg5[0:DV, :], in_=ctx_t[0][:, :])
                    nc.sync.dma_start(out=y[g * P : (g + 1) * P, :], in_=dbg5)
                    continue
                # ---- epilogue (ACT-free): 1/l, normalize, out-projection ----
                ctxn = []
                for hg in range(2):
                    # evacuate ctx to SBUF promptly (frees the PSUM buffer for
                    # the next chunk) -- rows 0..63 are ctx, row 64 is l.
                    ctxs = misc.tile(
                        [DV, NPAIR * P], F32, tag="ctxs", bufs=2, name=f"cs{g}{hg}"
                    )
                    nc.vector.tensor_copy(out=ctxs, in_=ctx_t[hg])
                    # engines are lane-locked; only DMA can move the l row from
                    # partition 64 to partition 0.
                    lr = misc.tile([1, NPAIR * P], F32, tag="lr", bufs=2, name=f"lr{g}{hg}")
                    nc.sync.dma_start(out=lr[0:1, :], in_=ctxs[D:DV, :])
                    linv = misc.tile(
                        [1, NPAIR * P], F32, tag="linv", bufs=2, name=f"li{g}{hg}"
                    )
                    nc.vector.reciprocal_approx_fast(
                        out=linv[0:1, :], in_=lr[0:1, :]
                    )
                    bc = sc_p.tile([D, NPAIR * P], F32, tag="sc", name=f"bc{g}{hg}")
                    for lo, n in ((0, 512), (512, 256)):  # bank-aligned pieces
                        nc.tensor.matmul(
                            out=bc[:, lo : lo + n],
                            lhsT=ones_sb[0:1, 0:D],
                            rhs=linv[0:1, lo : lo + n],
                            start=True,
                            stop=True,
                        )
                    cn = misc.tile(
                        [D, NPAIR * P], BF16, tag="ctxn", bufs=2, name=f"cn{g}{hg}"
                    )
                    nc.vector.tensor_mul(out=cn, in0=ctxs[0:D, :], in1=bc)
                    ctxn.append(cn)
                for fh in range(2):
                    op = sc_p.tile([P, 384], F32, tag="sc", name=f"op{g}{fh}")
                    for h in range(H):
                        nc.tensor.matmul(
                            out=op,
                            lhsT=ctxn[h // 6][:, (h % 6) * P : (h % 6 + 1) * P],
                            rhs=wob[:, h, fh * 384 : (fh + 1) * 384],
                            start=(h == 0),
                            stop=(h == H - 1),
                        )
                    outs = misc.tile([P, 384], F32, tag="outs", bufs=2, name=f"ou{g}{fh}")
                    nc.vector.tensor_add(
                        out=outs, in0=op, in1=bob_sb[:, fh * 384 : (fh + 1) * 384]
                    )
                    nc.sync.dma_start(
                        out=y[g * P : (g + 1) * P, fh * 384 : (fh + 1) * 384], in_=outs
                    )

    nc.compile()
    return nc


_NC_CACHE = None


def _get_program():
    global _NC_CACHE
    if _NC_CACHE is None:
        _NC_CACHE = build_program()
    return _NC_CACHE


def _host_inputs(x, Wq, Wk, Wv, Wo, bo):
    """Build per-core input maps (host does dtype casts + transposes only)."""
    BF = ml_dtypes.bfloat16
    x = np.ascontiguousarray(x.reshape(S, E), dtype=np.float32)
    # x^T in [P, NSG, EC, 512] layout: xt[p, sg, c, j] = x[sg*512 + j, c*128 + p]
    xt_full = np.ascontiguousarray(
        x.reshape(NSG, 512, EC, P).transpose(3, 0, 2, 1).astype(BF)
    )

    def w_arrange(W):
        # [P, EC, E]: w[p, c, f] = W[c*128 + p, f]
        return np.ascontiguousarray(
            np.asarray(W, dtype=np.float32).reshape(EC, P, E).transpose(1, 0, 2).astype(BF)
        )

    wq_a, wk_a, wv_a = w_arrange(Wq), w_arrange(Wk), w_arrange(Wv)
    wo_a = np.ascontiguousarray(
        np.asarray(Wo, dtype=np.float32).reshape(H, D, E).transpose(1, 0, 2).astype(BF)
    )
    bob = np.ascontiguousarray(np.broadcast_to(bo.astype(np.float32), (P, E)))
    # diagonal mask, replicated per head lane: [k, hl*128 + q] = k <= q
    tri = (np.arange(P)[:, None] <= np.arange(P)[None, :]).astype(np.float32)
    dmask = np.ascontiguousarray(np.tile(tri, (1, NPAIR)).astype(BF))

    # x^T columns for own q chunks: [P, EC, QOWN]
    xT = x.T.reshape(EC, P, S)  # [c, p, s]

    in_maps = []
    for c in range(N_CORES):
        chunks = [8 * g + c for g in range(NCH)]
        xqt = np.empty((P, EC, QOWN), dtype=BF)
        for g, gc in enumerate(chunks):
            xqt[:, :, g * P : (g + 1) * P] = (
                xT[:, :, gc * P : (gc + 1) * P].transpose(1, 0, 2).astype(BF)
            )
        btab = np.zeros((P, TOT_SLOTS), dtype=np.float32)
        for g in range(NCH):
            diagk = chunks[g]
            for s in range(SLOTS[g]):
                if s == SLOTS[g] - 1 or s < diagk:
                    v = 0.0  # diagonal slot or fully-valid block
                else:
                    v = NEG  # causally dead block
                btab[:, SLOT_BASE[g] + s] = v
        in_maps.append(
            {
                "xt": xt_full,
                "xqt": np.ascontiguousarray(xqt),
                "wq": wq_a,
                "wk": wk_a,
                "wv": wv_a,
                "wo": wo_a,
                "bob": bob,
                "btab": btab,
                "dmask": dmask,
            }
        )
    return in_maps


def kernel(x, Wq, Wk, Wv, Wo, bo, mask=None, **_ignored):
    nc = _get_program()
    in_maps = _host_inputs(
        np.asarray(x), np.asarray(Wq), np.asarray(Wk), np.asarray(Wv),
        np.asarray(Wo), np.asarray(bo),
    )
    trace = bool(int(os.environ.get("BASS_KERNEL_TRACE", "0")))
    res = bass_utils.run_bass_kernel_spmd(
        nc, in_maps, core_ids=list(range(N_CORES)), trace=trace
    )
    if trace:
        kernel.last_results = res
    out = np.empty((S, E), dtype=np.float32)
    for c in range(N_CORES):
        yc = res.results[c]["y"]
        for g in range(NCH):
            gc = 8 * g + c
            out[gc * P : (gc + 1) * P] = yc[g * P : (g + 1) * P]
    return out.reshape(1, S, E)


# revision 9
# speedup vs baseline: 2.5100x; 1.2494x over previous
"""Trainium2 Bass kernel for nn_MultiHeadAttention (B=1, S=4096, E=768, H=12, D=64).

Causal multi-head attention, sequence-parallel across 8 NeuronCores.

Strategy (single SPMD program, per-core variation is data-only):
- Query rows are split into 32 global chunks of 128 rows. Core c owns chunks
  G(g) = 8g + c for g = 0..3. Chunk g runs a fixed slot loop of 8(g+1)
  k-blocks (uniform across cores); causally-dead slots are killed by a
  per-slot exp bias of -60 (data), and the diagonal block is handled in the
  last slot with separately-projected "own" K/V tiles plus a triangular mask.
- K/V are sequence-sharded: each core projects K^T/V only for its own 512
  q-rows (which double as its K/V shard). Pieces 1-3 (rows 1024:4096) are
  exchanged via three pipelined 8-core AllGathers (HBM bounce buffers,
  TOPSP/SDMA do the work while the engines compute); piece 0 (rows 0:1024)
  is cheaper to recompute locally than to wait for a gather.
- Scores are computed transposed (S^T[k, q], k on partitions) so the exp
  output P^T feeds the attn@V matmul directly. Row sums l come from a fused
  ones-column appended to V. The epilogue is ACT-free (DVE
  reciprocal_approx_fast + K=1 broadcast matmul + DVE multiply), so the
  scalar engine runs nothing but the exp stream - one table set, loaded
  once, and exp is the slot-loop bottleneck at (768+352)/1.2 ns x2 per slot.
- Epilogues are deferred: chunk g's normalization/out-projection instructions
  are emitted after slot 3 of chunk g+1, so the ACT exp stream and PE slot
  stream never drain while the epilogue's serial chain resolves. ctx PSUM is
  evacuated to SBUF immediately at chunk end to release the accumulators.
- All matmuls bf16 (fp32 PSUM). x arrives host-transposed/bf16; weights
  arrive bf16 pre-arranged.
"""

import os
from contextlib import ExitStack

import numpy as np
import ml_dtypes

import concourse.bass as bass
import concourse.tile as tile
from concourse import bacc, bass_utils, mybir

F32 = mybir.dt.float32
BF16 = mybir.dt.bfloat16

N_CORES = 8
S, E, H, D = 4096, 768, 12, 64
P = 128
NCH = 4  # chunks per core
SLOTS = [8, 16, 24, 32]  # slot count for chunk g
SLOT_BASE = [0, 8, 24, 48]  # cumulative
TOT_SLOTS = 80
EC = E // P  # 6 e-chunks of 128
NPAIR = 6  # head pairs
QOWN = NCH * P  # 512 own q rows
NEG = -60.0  # exp bias for masked slots: exp(-60 +- 4) == 0 numerically
DV = D + 1  # v row block: 64 dims + ones col
AGW = NPAIR * P + H * DV  # 768 + 780 = 1548 allgather row width


def build_program():
    nc = bacc.Bacc("TRN2", target_bir_lowering=False, debug=False, num_devices=N_CORES)

    # host-prepped inputs (bf16, pre-transposed / pre-arranged)
    xt0 = nc.dram_tensor("xt0", [P, 2, EC, 512], BF16, kind="ExternalInput").ap()
    xqt = nc.dram_tensor("xqt", [P, EC, QOWN], BF16, kind="ExternalInput").ap()
    wq = nc.dram_tensor("wq", [P, EC, E], BF16, kind="ExternalInput").ap()
    wk = nc.dram_tensor("wk", [P, EC, E], BF16, kind="ExternalInput").ap()
    wv = nc.dram_tensor("wv", [P, EC, E], BF16, kind="ExternalInput").ap()
    wo = nc.dram_tensor("wo", [D, H, E], BF16, kind="ExternalInput").ap()
    bob = nc.dram_tensor("bob", [P, E], F32, kind="ExternalInput").ap()
    btab = nc.dram_tensor("btab", [P, TOT_SLOTS], F32, kind="ExternalInput").ap()
    dmask = nc.dram_tensor("dmask", [P, NPAIR * P], BF16, kind="ExternalInput").ap()
    y = nc.dram_tensor("y", [QOWN, E], F32, kind="ExternalOutput").ap()

    with tile.TileContext(nc) as tc, ExitStack() as top:
        const = top.enter_context(tc.tile_pool(name="const", bufs=1))
        big = top.enter_context(tc.tile_pool(name="big", bufs=1))
        own = top.enter_context(tc.tile_pool(name="own", bufs=1))
        dram = top.enter_context(tc.tile_pool(name="dram", bufs=1, space="DRAM"))
        sc_p = top.enter_context(tc.tile_pool(name="scp", bufs=2, space="PSUM"))
        ctx_p = top.enter_context(tc.tile_pool(name="ctxp", bufs=1, space="PSUM"))
        epi_p = top.enter_context(tc.tile_pool(name="epip", bufs=1, space="PSUM"))

        # ---- constants ----
        btab_sb = const.tile([P, TOT_SLOTS], F32, tag="btab")
        nc.sync.dma_start(out=btab_sb, in_=btab)
        dmask_bf = const.tile([P, NPAIR * P], BF16, tag="dmaskb")
        nc.sync.dma_start(out=dmask_bf, in_=dmask)
        bob_sb = const.tile([P, E], F32, tag="bob")
        nc.sync.dma_start(out=bob_sb, in_=bob)
        ones_sb = const.tile([P, D], F32, tag="ones")
        nc.vector.memset(ones_sb, 1.0)
        zb = const.tile([P, 512], BF16, tag="zb")
        nc.vector.memset(zb, 0.0)

        # ---- persistent bf16 operands (K^T / ones+V in 4 gather pieces) ----
        ktp = [
            big.tile([P, NPAIR, 1024], BF16, tag=f"kt{j}", name=f"kt{j}")
            for j in range(4)
        ]
        vtp = [
            big.tile([P, 8, H * DV], BF16, tag=f"vt{j}", name=f"vt{j}")
            for j in range(4)
        ]
        vtp_v = [t.rearrange("p b (h c) -> p b h c", c=DV) for t in vtp]
        qtp = own.tile([P, NPAIR, 2, QOWN], BF16, tag="qtp")
        nc.vector.memset(qtp[D:P, :, 0, :], 0.0)
        nc.vector.memset(qtp[0:D, :, 1, :], 0.0)
        kto = own.tile([P, NPAIR, QOWN], BF16, tag="kto")  # own K^T (diagonal + AG src)
        vto = own.tile([P, NCH, H * DV], BF16, tag="vto")  # own ones+V
        vto_v = vto.rearrange("p b (h c) -> p b h c", c=DV)
        nc.vector.memset(vto_v[:, :, :, D : D + 1], 1.0)
        for j in range(4):
            nc.vector.memset(vtp_v[j][:, :, :, D : D + 1], 1.0)

        # ================= projection phase =================
        with ExitStack() as proj:
            wpool = proj.enter_context(tc.tile_pool(name="wpool", bufs=1))
            xq_pool = proj.enter_context(tc.tile_pool(name="xqp", bufs=1))
            xt_p = proj.enter_context(tc.tile_pool(name="xtp", bufs=2))

            wqb = wpool.tile([P, EC, E], BF16, tag="wqb")
            wkb = wpool.tile([P, EC, E], BF16, tag="wkb")
            wvb = wpool.tile([P, EC, E], BF16, tag="wvb")
            nc.sync.dma_start(out=wkb, in_=wk)
            nc.sync.dma_start(out=wvb, in_=wv)
            nc.sync.dma_start(out=wqb, in_=wq)

            xqt_sb = xq_pool.tile([P, EC, QOWN], BF16, tag="xqt")
            nc.sync.dma_start(out=xqt_sb, in_=xqt)

            def project_pairs(w_bf, dst, src_t, ncols, split=False):
                for pc in range(NPAIR):
                    ps = sc_p.tile([P, ncols], F32, tag="sc", name=f"ps_p{pc}")
                    for ec in range(EC):
                        nc.tensor.matmul(
                            out=ps,
                            lhsT=w_bf[:, ec, pc * P : (pc + 1) * P],
                            rhs=src_t[:, ec, :],
                            start=(ec == 0),
                            stop=(ec == EC - 1),
                        )
                    if split:
                        nc.vector.tensor_copy(out=dst[0:D, pc, 0, :], in_=ps[0:D, :])
                        nc.vector.tensor_copy(out=dst[D:P, pc, 1, :], in_=ps[D:P, :])
                    else:
                        nc.scalar.copy(out=dst[:, pc, :], in_=ps)

            def project_v(dst_view, xt_tile, xt_col0):
                for half in range(2):
                    ps = sc_p.tile([P, E // 2], F32, tag="sc", name=f"ps_v{half}")
                    for ec in range(EC):
                        nc.tensor.matmul(
                            out=ps,
                            lhsT=xt_tile[:, ec, xt_col0 : xt_col0 + P],
                            rhs=wvb[:, ec, half * 384 : (half + 1) * 384],
                            start=(ec == 0),
                            stop=(ec == EC - 1),
                        )
                    nc.vector.tensor_copy(
                        out=dst_view[:, half * 6 : (half + 1) * 6, 0:D],
                        in_=ps.rearrange("p (h d) -> p h d", d=D),
                    )

            # own-shard K/V first: they feed the AllGathers
            project_pairs(wkb, kto, xqt_sb, QOWN)
            for qb in range(NCH):
                project_v(vto_v[:, qb, :, :], xqt_sb, qb * P)

            # bounce own chunks 1-3 out and trigger the three AllGathers
            ag_in = [
                dram.tile([P, AGW], BF16, tag=f"agi{j}", name=f"agi{j}")
                for j in range(1, 4)
            ]
            ag_out = [
                dram.tile(
                    [N_CORES * P, AGW], BF16, tag=f"ago{j}", name=f"ago{j}",
                    addr_space="Shared",
                )
                for j in range(1, 4)
            ]
            for j in range(1, 4):
                bi = ag_in[j - 1]
                nc.gpsimd.dma_start(
                    out=bi[:, 0 : NPAIR * P].rearrange("p (c q) -> p c q", q=P),
                    in_=kto[:, :, j * P : (j + 1) * P],
                )
                nc.gpsimd.dma_start(out=bi[:, NPAIR * P : AGW], in_=vto[:, j, :])
                nc.gpsimd.collective_compute(
                    "AllGather",
                    mybir.AluOpType.bypass,
                    replica_groups=[list(range(N_CORES))],
                    ins=[bi[:].opt()],
                    outs=[ag_out[j - 1][:].opt()],
                )

            # own Q while the gathers fly
            project_pairs(wqb, qtp, xqt_sb, QOWN, split=True)

            # piece 0 (rows 0:1024) recomputed locally - cheaper than waiting
            for sg in range(2):
                xt_g = xt_p.tile([P, EC, 512], BF16, tag="xtg", name=f"xt_{sg}")
                nc.sync.dma_start(out=xt_g, in_=xt0[:, sg, :, :])
                for pc in range(NPAIR):
                    ps = sc_p.tile([P, 512], F32, tag="sc", name=f"ps_k{pc}")
                    for ec in range(EC):
                        nc.tensor.matmul(
                            out=ps,
                            lhsT=wkb[:, ec, pc * P : (pc + 1) * P],
                            rhs=xt_g[:, ec, :],
                            start=(ec == 0),
                            stop=(ec == EC - 1),
                        )
                    nc.scalar.copy(
                        out=ktp[0][:, pc, sg * 512 : (sg + 1) * 512], in_=ps
                    )
                for b in range(4):
                    project_v(vtp_v[0][:, 4 * sg + b, :, :], xt_g, b * P)

            # scatter the gathered pieces into SBUF as they complete
            for j in range(1, 4):
                ao = ag_out[j - 1]
                for r in range(N_CORES):
                    nc.gpsimd.dma_start(
                        out=ktp[j][:, :, r * P : (r + 1) * P],
                        in_=ao[r * P : (r + 1) * P, 0 : NPAIR * P].rearrange(
                            "p (c q) -> p c q", q=P
                        ),
                    )
                    nc.gpsimd.dma_start(
                        out=vtp[j][:, r, :], in_=ao[r * P : (r + 1) * P, NPAIR * P : AGW]
                    )

        phase = os.environ.get("KERNEL_PHASE", "full")
        n_chunks_run = 0 if phase == "proj" else NCH
        if phase.startswith("att"):
            n_chunks_run = int(phase[3:])

        # ================= attention phase =================
        with ExitStack() as att:
            wo_pool = att.enter_context(tc.tile_pool(name="wop", bufs=1))
            wob = wo_pool.tile([D, H, E], BF16, tag="wob")
            nc.sync.dma_start(out=wob, in_=wo)

            pt_p = att.enter_context(tc.tile_pool(name="pt", bufs=1))
            misc = att.enter_context(tc.tile_pool(name="misc", bufs=1))

            def make_epilogue(g, ctxs):
                """Deferred normalization + out-projection for chunk g, split
                into three emission parts so the PE/ACT slot streams never
                drain while the serial chain resolves. ctxs is the SBUF copy
                [DV, 2*NPAIR*P]; row 64 is l, rows 0:64 are ctx."""
                ctxn = []

                def part_a():
                    # 1/l and the per-partition broadcast + normalize
                    lr = misc.tile(
                        [1, 2 * NPAIR * P], F32, tag="lr", bufs=2, name=f"lr{g}"
                    )
                    nc.sync.dma_start(out=lr[0:1, :], in_=ctxs[D:DV, :])
                    linv = misc.tile(
                        [1, 2 * NPAIR * P], F32, tag="linv", bufs=2, name=f"li{g}"
                    )
                    nc.vector.reciprocal_approx_fast(out=linv[0:1, :], in_=lr[0:1, :])
                    for hg in range(2):
                        cn = misc.tile(
                            [D, NPAIR * P], BF16, tag=f"ctxn{hg}", bufs=2,
                            name=f"cn{g}{hg}",
                        )
                        for lo, n in ((0, 512), (512, 256)):  # 1-bank pieces
                            bc = epi_p.tile(
                                [D, n], F32, tag="epi", name=f"bc{g}{hg}{lo}"
                            )
                            nc.tensor.matmul(
                                out=bc,
                                lhsT=ones_sb[0:1, 0:D],
                                rhs=linv[0:1, hg * 768 + lo : hg * 768 + lo + n],
                                start=True,
                                stop=True,
                            )
                            nc.vector.tensor_mul(
                                out=cn[:, lo : lo + n],
                                in0=ctxs[0:D, hg * 768 + lo : hg * 768 + lo + n],
                                in1=bc,
                            )
                        ctxn.append(cn)

                def out_proj(fh):
                    def emit():
                        op = epi_p.tile([P, 384], F32, tag="epi", name=f"op{g}{fh}")
                        for h in range(H):
                            nc.tensor.matmul(
                                out=op,
                                lhsT=ctxn[h // 6][:, (h % 6) * P : (h % 6 + 1) * P],
                                rhs=wob[:, h, fh * 384 : (fh + 1) * 384],
                                start=(h == 0),
                                stop=(h == H - 1),
                            )
                        outs = misc.tile(
                            [P, 384], F32, tag="outs", bufs=2, name=f"ou{g}{fh}"
                        )
                        nc.vector.tensor_add(
                            out=outs, in0=op, in1=bob_sb[:, fh * 384 : (fh + 1) * 384]
                        )
                        nc.sync.dma_start(
                            out=y[g * P : (g + 1) * P, fh * 384 : (fh + 1) * 384],
                            in_=outs,
                        )

                    return emit

                return [part_a, out_proj(0), out_proj(1)]

            pending_epi = []
            for g in range(n_chunks_run):
                nslot = SLOTS[g]
                # single merged ctx accumulator for both head groups: [65, 1536]
                ctx_t = ctx_p.tile(
                    [DV, 2 * NPAIR * P], F32, tag="ctx", name=f"ctx_g{g}"
                )

                def ctx_fence(start):
                    # bank-wide zero matmuls fencing the per-head accumulation:
                    # one start=True / stop=True group per PSUM bank, with all
                    # real ctx matmuls as flags=0 accumulates in between. The
                    # full-bank writes give WAW deps ordering them correctly.
                    for lo in range(0, 2 * NPAIR * P, 512):
                        nc.tensor.matmul(
                            out=ctx_t[0:DV, lo : lo + 512],
                            lhsT=zb[0:1, 0:DV],
                            rhs=zb[0:1, 0:512],
                            start=start,
                            stop=not start,
                        )

                def emit_scores(s):
                    """Scores + exp for slot s; returns the two pt tiles."""
                    is_diag = s == nslot - 1
                    pj, pcol = s >> 3, s & 7
                    pts = []
                    for hg in range(2):
                        sc = sc_p.tile(
                            [P, NPAIR * P], F32, tag="sc", name=f"sc_{g}_{s}_{hg}"
                        )
                        for pl in range(3):
                            pc = hg * 3 + pl
                            if is_diag:
                                lhsT = kto[:, pc, g * P : (g + 1) * P]
                            else:
                                lhsT = ktp[pj][:, pc, pcol * P : (pcol + 1) * P]
                            # single-shot scores; interleaved start=True groups in
                            # one bank are HW-safe (per-element data unaffected).
                            # rhs [128, 2, 128] = both zero-padded Q variants ->
                            # out [128, 256] = both heads of the pair.
                            nc.tensor.matmul(
                                out=sc[:, pl * 2 * P : (pl + 1) * 2 * P],
                                lhsT=lhsT,
                                rhs=qtp[:, pc, :, g * P : (g + 1) * P],
                                start=True,
                                stop=True,
                                skip_group_check=True,
                            )
                        pt = pt_p.tile(
                            [P, NPAIR * P], BF16, tag="pt", bufs=3,
                            name=f"pt_{g}_{s}_{hg}",
                        )
                        sg_idx = SLOT_BASE[g] + s
                        nc.scalar.activation(
                            out=pt,
                            in_=sc,
                            func=mybir.ActivationFunctionType.Exp,
                            bias=btab_sb[:, sg_idx : sg_idx + 1],
                            scale=0.125,
                        )
                        if is_diag:
                            ptm = pt_p.tile(
                                [P, NPAIR * P], BF16, tag="ptm", name=f"ptm_{g}_{hg}"
                            )
                            nc.vector.tensor_mul(out=ptm, in0=pt, in1=dmask_bf)
                            pt = ptm
                        pts.append(pt)
                    return pts

                def emit_attnv(s, pts):
                    is_diag = s == nslot - 1
                    pj, pcol = s >> 3, s & 7
                    for hg in range(2):
                        for hl in range(6):
                            h = hg * 6 + hl
                            vsrc = (
                                vto_v[:, g, h, :]
                                if is_diag
                                else vtp_v[pj][:, pcol, h, :]
                            )
                            nc.tensor.matmul(
                                out=ctx_t[:, (hg * 6 + hl) * P : (hg * 6 + hl + 1) * P],
                                lhsT=vsrc,
                                rhs=pts[hg][:, hl * P : (hl + 1) * P],
                                start=False,
                                stop=False,
                            )

                # slots 0/1's scores+exp go ahead of the fence: the fence's
                # WAW on the ctx buffer waits for the previous chunk's SBUF
                # evacuation, and ACT must stay fed across that hop.
                pts0 = emit_scores(0)
                pts1 = emit_scores(1)
                ctx_fence(start=True)
                emit_attnv(0, pts0)
                if pending_epi:
                    pending_epi[0]()  # normalize prev chunk
                emit_attnv(1, pts1)
                for s in range(2, nslot):
                    pts = emit_scores(s)
                    if pending_epi and s == 3:
                        pending_epi[1]()  # out-proj half 0
                    if pending_epi and s == 5:
                        pending_epi[2]()  # out-proj half 1
                        pending_epi = []
                    emit_attnv(s, pts)
                ctx_fence(start=False)
                # evacuate ctx to SBUF immediately: releases the PSUM
                # accumulator so the next chunk's fence isn't blocked.
                ctxs = misc.tile(
                    [DV, 2 * NPAIR * P], F32, tag="ctxs", bufs=2, name=f"cs{g}"
                )
                nc.vector.tensor_copy(out=ctxs, in_=ctx_t)
                pending_epi = make_epilogue(g, ctxs)
            for part in pending_epi:
                part()

    nc.compile()
    return nc


_NC_CACHE = None


def _get_program():
    global _NC_CACHE
    if _NC_CACHE is None:
        _NC_CACHE = build_program()
    return _NC_CACHE


def _host_inputs(x, Wq, Wk, Wv, Wo, bo):
    """Build per-core input maps (host does dtype casts + transposes only)."""
    BF = ml_dtypes.bfloat16
    x = np.ascontiguousarray(x.reshape(S, E), dtype=np.float32)
    # x^T piece 0 in [P, 2, EC, 512] layout: [p, sg, c, j] = x[sg*512+j, c*128+p]
    xt0 = np.ascontiguousarray(
        x[0:1024].reshape(2, 512, EC, P).transpose(3, 0, 2, 1).astype(BF)
    )

    def w_arrange(W):
        return np.ascontiguousarray(
            np.asarray(W, dtype=np.float32).reshape(EC, P, E).transpose(1, 0, 2).astype(BF)
        )

    wq_a, wk_a, wv_a = w_arrange(Wq), w_arrange(Wk), w_arrange(Wv)
    wo_a = np.ascontiguousarray(
        np.asarray(Wo, dtype=np.float32).reshape(H, D, E).transpose(1, 0, 2).astype(BF)
    )
    bob = np.ascontiguousarray(np.broadcast_to(bo.astype(np.float32), (P, E)))
    tri = (np.arange(P)[:, None] <= np.arange(P)[None, :]).astype(np.float32)
    dmask = np.ascontiguousarray(np.tile(tri, (1, NPAIR)).astype(BF))

    xT = x.T.reshape(EC, P, S)  # [c, p, s]

    in_maps = []
    for c in range(N_CORES):
        chunks = [8 * g + c for g in range(NCH)]
        xqt = np.empty((P, EC, QOWN), dtype=BF)
        for g, gc in enumerate(chunks):
            xqt[:, :, g * P : (g + 1) * P] = (
                xT[:, :, gc * P : (gc + 1) * P].transpose(1, 0, 2).astype(BF)
            )
        btab = np.zeros((P, TOT_SLOTS), dtype=np.float32)
        for g in range(NCH):
            diagk = chunks[g]
            for s in range(SLOTS[g]):
                if s == SLOTS[g] - 1 or s < diagk:
                    v = 0.0  # diagonal slot or fully-valid block
                else:
                    v = NEG  # causally dead block
                btab[:, SLOT_BASE[g] + s] = v
        in_maps.append(
            {
                "xt0": xt0,
                "xqt": np.ascontiguousarray(xqt),
                "wq": wq_a,
                "wk": wk_a,
                "wv": wv_a,
                "wo": wo_a,
                "bob": bob,
                "btab": btab,
                "dmask": dmask,
            }
        )
    return in_maps


def kernel(x, Wq, Wk, Wv, Wo, bo, mask=None, **_ignored):
    nc = _get_program()
    in_maps = _host_inputs(
        np.asarray(x), np.asarray(Wq), np.asarray(Wk), np.asarray(Wv),
        np.asarray(Wo), np.asarray(bo),
    )
    trace = bool(int(os.environ.get("BASS_KERNEL_TRACE", "0")))
    res = bass_utils.run_bass_kernel_spmd(
        nc, in_maps, core_ids=list(range(N_CORES)), trace=trace
    )
    if trace:
        kernel.last_results = res
    out = np.empty((S, E), dtype=np.float32)
    for c in range(N_CORES):
        yc = res.results[c]["y"]
        for g in range(NCH):
            gc = 8 * g + c
            out[gc * P : (gc + 1) * P] = yc[g * P : (g + 1) * P]
    return out.reshape(1, S, E)


# revision 12
# speedup vs baseline: 2.5823x; 1.0288x over previous
"""Trainium2 Bass kernel for nn_MultiHeadAttention (B=1, S=4096, E=768, H=12, D=64).

Causal multi-head attention, sequence-parallel across 8 NeuronCores.

Strategy (single SPMD program, per-core variation is data-only):
- Query rows are split into 32 global chunks of 128 rows. Core c owns chunks
  G(g) = 8g + c for g = 0..3. Chunk g runs a fixed slot loop of 8(g+1)
  k-blocks (uniform across cores); causally-dead slots are killed by a
  per-slot exp bias of -60 (data), and the diagonal block is handled in the
  last slot with separately-projected "own" K/V tiles plus a triangular mask.
- K/V are sequence-sharded: each core projects K^T/V only for its own 512
  q-rows (which double as its K/V shard). Pieces 1-3 (rows 1024:4096) are
  exchanged via three pipelined 8-core AllGathers (HBM bounce buffers,
  TOPSP/SDMA move the data while the engines compute); piece 0 (rows 0:1024)
  is cheaper to recompute locally than to wait for a gather, and its second
  half is interleaved after chunk 0's first slots so the exp stream starts
  as early as possible.
- Scores are computed transposed (S^T[k, q], k on partitions) so the exp
  output P^T feeds the attn@V matmul directly. Row sums l come from a fused
  ones-column appended to V. The epilogue is ACT-free (DVE
  reciprocal_approx_fast + K=1 broadcast matmul + DVE multiply): the scalar
  engine runs nothing but the exp stream - one table set loaded once - and
  exp is the slot-loop bottleneck at 2 x (768+352)/1.2 ns per slot.
- Epilogues are deferred and split: the DVE/DMA-only reciprocal chain is
  emitted at the next chunk's start, and the PE pieces (broadcast, out-proj
  halves) at slots 4/6/8 of the next chunk, when their inputs are guaranteed
  resolved - so the in-order PE queue never parks on a not-yet-ready
  epilogue matmul and the HAM clock gate stays open.
"""

import os
from contextlib import ExitStack

import numpy as np
import ml_dtypes

import concourse.bass as bass
import concourse.tile as tile
from concourse import bacc, bass_utils, mybir

F32 = mybir.dt.float32
BF16 = mybir.dt.bfloat16

N_CORES = 8
S, E, H, D = 4096, 768, 12, 64
P = 128
NCH = 4  # chunks per core
SLOTS = [8, 16, 24, 32]  # slot count for chunk g
SLOT_BASE = [0, 8, 24, 48]  # cumulative
TOT_SLOTS = 80
EC = E // P  # 6 e-chunks of 128
NPAIR = 6  # head pairs
QOWN = NCH * P  # 512 own q rows
NEG = -60.0  # exp bias for masked slots: exp(-60 +- 4) == 0 numerically
DV = D + 1  # v row block: 64 dims + ones col
AGW = NPAIR * P + H * DV  # 768 + 780 = 1548 allgather row width


def build_program():
    nc = bacc.Bacc("TRN2", target_bir_lowering=False, debug=False, num_devices=N_CORES)

    # host-prepped inputs (bf16, pre-transposed / pre-arranged)
    xt0 = nc.dram_tensor("xt0", [P, 2, EC, 512], BF16, kind="ExternalInput").ap()
    xqt = nc.dram_tensor("xqt", [P, EC, QOWN], BF16, kind="ExternalInput").ap()
    wq = nc.dram_tensor("wq", [P, EC, E], BF16, kind="ExternalInput").ap()
    wk = nc.dram_tensor("wk", [P, EC, E], BF16, kind="ExternalInput").ap()
    wv = nc.dram_tensor("wv", [P, EC, E], BF16, kind="ExternalInput").ap()
    wo = nc.dram_tensor("wo", [D, H, E], BF16, kind="ExternalInput").ap()
    bob = nc.dram_tensor("bob", [P, E], F32, kind="ExternalInput").ap()
    btab = nc.dram_tensor("btab", [P, TOT_SLOTS], F32, kind="ExternalInput").ap()
    dmask = nc.dram_tensor("dmask", [P, NPAIR * P], BF16, kind="ExternalInput").ap()
    y = nc.dram_tensor("y", [QOWN, E], F32, kind="ExternalOutput").ap()

    with tile.TileContext(nc) as tc, ExitStack() as top:
        const = top.enter_context(tc.tile_pool(name="const", bufs=1))
        big = top.enter_context(tc.tile_pool(name="big", bufs=1))
        own = top.enter_context(tc.tile_pool(name="own", bufs=1))
        kvw = top.enter_context(tc.tile_pool(name="kvw", bufs=1))
        xt_p = top.enter_context(tc.tile_pool(name="xtp", bufs=2))
        dram = top.enter_context(tc.tile_pool(name="dram", bufs=1, space="DRAM"))
        sc_p = top.enter_context(tc.tile_pool(name="scp", bufs=2, space="PSUM"))
        ctx_p = top.enter_context(tc.tile_pool(name="ctxp", bufs=1, space="PSUM"))
        epi_p = top.enter_context(tc.tile_pool(name="epip", bufs=1, space="PSUM"))

        # K/V weights live for the whole kernel (piece-0 second half is
        # projected inside the attention loop); wk + xqt are DMA'd FIRST so
        # the own-shard K projection starts as early as possible.
        wkb = kvw.tile([P, EC, E], BF16, tag="wkb")
        nc.sync.dma_start(out=wkb, in_=wk)
        wvb = kvw.tile([P, EC, E], BF16, tag="wvb")
        nc.sync.dma_start(out=wvb, in_=wv)

        # ---- persistent operands ----
        ktp = [
            big.tile([P, NPAIR, 1024], BF16, tag=f"kt{j}", name=f"kt{j}")
            for j in range(4)
        ]
        vtp = [
            big.tile([P, 8, H * DV], BF16, tag=f"vt{j}", name=f"vt{j}")
            for j in range(4)
        ]
        vtp_v = [t.rearrange("p b (h c) -> p b h c", c=DV) for t in vtp]
        qtp = own.tile([P, NPAIR, 2, QOWN], BF16, tag="qtp")
        nc.vector.memset(qtp[D:P, :, 0, :], 0.0)
        nc.vector.memset(qtp[0:D, :, 1, :], 0.0)
        kto = own.tile([P, NPAIR, QOWN], BF16, tag="kto")  # own K^T (diag + AG src)
        vto = own.tile([P, NCH, H * DV], BF16, tag="vto")  # own ones+V
        vto_v = vto.rearrange("p b (h c) -> p b h c", c=DV)
        nc.vector.memset(vto_v[:, :, :, D : D + 1], 1.0)
        for j in range(4):
            nc.vector.memset(vtp_v[j][:, :, :, D : D + 1], 1.0)

        ones_sb = const.tile([P, D], F32, tag="ones")
        nc.vector.memset(ones_sb, 1.0)
        zb = const.tile([P, 512], BF16, tag="zb")
        nc.vector.memset(zb, 0.0)

        def project_pairs(w_bf, dst, src_t, ncols, split=False):
            for pc in range(NPAIR):
                ps = sc_p.tile([P, ncols], F32, tag="sc", name=f"ps_p{pc}")
                for ec in range(EC):
                    nc.tensor.matmul(
                        out=ps,
                        lhsT=w_bf[:, ec, pc * P : (pc + 1) * P],
                        rhs=src_t[:, ec, :],
                        start=(ec == 0),
                        stop=(ec == EC - 1),
                    )
                if split:
                    nc.vector.tensor_copy(out=dst[0:D, pc, 0, :], in_=ps[0:D, :])
                    nc.vector.tensor_copy(out=dst[D:P, pc, 1, :], in_=ps[D:P, :])
                else:
                    nc.scalar.copy(out=dst[:, pc, :], in_=ps)

        def project_v(dst_view, xt_tile, xt_col0):
            for half in range(2):
                ps = sc_p.tile([P, E // 2], F32, tag="sc", name=f"ps_v{half}")
                for ec in range(EC):
                    nc.tensor.matmul(
                        out=ps,
                        lhsT=xt_tile[:, ec, xt_col0 : xt_col0 + P],
                        rhs=wvb[:, ec, half * 384 : (half + 1) * 384],
                        start=(ec == 0),
                        stop=(ec == EC - 1),
                    )
                nc.vector.tensor_copy(
                    out=dst_view[:, half * 6 : (half + 1) * 6, 0:D],
                    in_=ps.rearrange("p (h d) -> p h d", d=D),
                )

        def project_piece0(sg, xt_src):
            xt_g = xt_p.tile([P, EC, 512], BF16, tag="xtg", name=f"xt_{sg}")
            nc.sync.dma_start(out=xt_g, in_=xt_src)
            for pc in range(NPAIR):
                ps = sc_p.tile([P, 512], F32, tag="sc", name=f"p0k{sg}{pc}")
                for ec in range(EC):
                    nc.tensor.matmul(
                        out=ps,
                        lhsT=wkb[:, ec, pc * P : (pc + 1) * P],
                        rhs=xt_g[:, ec, :],
                        start=(ec == 0),
                        stop=(ec == EC - 1),
                    )
                nc.scalar.copy(out=ktp[0][:, pc, sg * 512 : (sg + 1) * 512], in_=ps)
            for b in range(4):
                project_v(vtp_v[0][:, 4 * sg + b, :, :], xt_g, b * P)

        # ================= projection phase =================
        with ExitStack() as proj:
            wq_pool = proj.enter_context(tc.tile_pool(name="wqp", bufs=1))
            xq_pool = proj.enter_context(tc.tile_pool(name="xqp", bufs=1))

            xqt_sb = xq_pool.tile([P, EC, QOWN], BF16, tag="xqt")
            nc.sync.dma_start(out=xqt_sb, in_=xqt)
            wqb = wq_pool.tile([P, EC, E], BF16, tag="wqb")
            nc.sync.dma_start(out=wqb, in_=wq)

            # own-shard K/V first: they feed the AllGathers
            project_pairs(wkb, kto, xqt_sb, QOWN)
            for qb in range(NCH):
                project_v(vto_v[:, qb, :, :], xqt_sb, qb * P)

            # bounce own chunks 1-3 out; trigger AG j then immediately queue
            # its scatter DMAs (they wait on the collective's completion).
            ag_in = [
                dram.tile([P, AGW], BF16, tag=f"agi{j}", name=f"agi{j}")
                for j in range(1, 4)
            ]
            ag_out = [
                dram.tile(
                    [N_CORES * P, AGW], BF16, tag=f"ago{j}", name=f"ago{j}",
                    addr_space="Shared",
                )
                for j in range(1, 4)
            ]
            for j in range(1, 4):
                bi = ag_in[j - 1]
                nc.gpsimd.dma_start(
                    out=bi[:, 0 : NPAIR * P].rearrange("p (c q) -> p c q", q=P),
                    in_=kto[:, :, j * P : (j + 1) * P],
                )
                nc.gpsimd.dma_start(out=bi[:, NPAIR * P : AGW], in_=vto[:, j, :])
            for j in range(1, 4):
                ao = ag_out[j - 1]
                nc.gpsimd.collective_compute(
                    "AllGather",
                    mybir.AluOpType.bypass,
                    replica_groups=[list(range(N_CORES))],
                    ins=[ag_in[j - 1][:].opt()],
                    outs=[ao[:].opt()],
                )
                for r in range(N_CORES):
                    nc.gpsimd.dma_start(
                        out=ktp[j][:, :, r * P : (r + 1) * P],
                        in_=ao[r * P : (r + 1) * P, 0 : NPAIR * P].rearrange(
                            "p (c q) -> p c q", q=P
                        ),
                    )
                    nc.gpsimd.dma_start(
                        out=vtp[j][:, r, :],
                        in_=ao[r * P : (r + 1) * P, NPAIR * P : AGW],
                    )

            # own Q while the gathers fly
            project_pairs(wqb, qtp, xqt_sb, QOWN, split=True)

            # piece 0 first half (rows 0:512); second half is interleaved
            # into chunk 0's slot loop below.
            project_piece0(0, xt0[:, 0, :, :])

        phase = os.environ.get("KERNEL_PHASE", "full")
        n_chunks_run = 0 if phase == "proj" else NCH
        if phase.startswith("att"):
            n_chunks_run = int(phase[3:])

        # ================= attention phase =================
        with ExitStack() as att:
            wo_pool = att.enter_context(tc.tile_pool(name="wop", bufs=1))
            wob = wo_pool.tile([D, H, E], BF16, tag="wob")
            nc.sync.dma_start(out=wob, in_=wo)
            btab_sb = const.tile([P, TOT_SLOTS], F32, tag="btab")
            nc.sync.dma_start(out=btab_sb, in_=btab)
            dmask_bf = const.tile([P, NPAIR * P], BF16, tag="dmaskb")
            nc.sync.dma_start(out=dmask_bf, in_=dmask)
            bob_sb = const.tile([P, E], F32, tag="bob")
            nc.sync.dma_start(out=bob_sb, in_=bob)

            pt_p = att.enter_context(tc.tile_pool(name="pt", bufs=1))
            misc = att.enter_context(tc.tile_pool(name="misc", bufs=1))

            def make_epilogue(g, ctxs, last=False):
                """Deferred epilogue for chunk g, as (slot, fn) emission parts.
                ctxs is the SBUF ctx copy [DV, 2*NPAIR*P]; row 64 is l."""
                lr = misc.tile([1, 2 * NPAIR * P], F32, tag="lr", bufs=1, name=f"lr{g}")
                linv = misc.tile(
                    [1, 2 * NPAIR * P], F32, tag="linv", bufs=1, name=f"li{g}"
                )
                ctxn = []

                def part_recip():
                    # DVE/DMA only - no PE instructions, so the PE queue never
                    # parks on this chain.
                    nc.sync.dma_start(out=lr[0:1, :], in_=ctxs[D:DV, :])
                    nc.vector.reciprocal_approx_fast(out=linv[0:1, :], in_=lr[0:1, :])

                def part_norm():
                    for hg in range(2):
                        cn = misc.tile(
                            [D, NPAIR * P], BF16, tag=f"ctxn{hg}", bufs=1,
                            name=f"cn{g}{hg}",
                        )
                        for lo, n in ((0, 512), (512, 256)):  # 1-bank pieces
                            bc = epi_p.tile(
                                [D, n], F32, tag="epi", name=f"bc{g}{hg}{lo}"
                            )
                            nc.tensor.matmul(
                                out=bc,
                                lhsT=ones_sb[0:1, 0:D],
                                rhs=linv[0:1, hg * 768 + lo : hg * 768 + lo + n],
                                start=True,
                                stop=True,
                            )
                            nc.vector.tensor_mul(
                                out=cn[:, lo : lo + n],
                                in0=ctxs[0:D, hg * 768 + lo : hg * 768 + lo + n],
                                in1=bc,
                            )
                        ctxn.append(cn)

                def out_proj(fh):
                    def emit():
                        if last:  # sc pool is free at the end: f0/f1 overlap
                            op = sc_p.tile([P, 384], F32, tag="sc", name=f"op{g}{fh}")
                        else:
                            op = epi_p.tile([P, 384], F32, tag="epi", name=f"op{g}{fh}")
                        for h in range(H):
                            nc.tensor.matmul(
                                out=op,
                                lhsT=ctxn[h // 6][:, (h % 6) * P : (h % 6 + 1) * P],
                                rhs=wob[:, h, fh * 384 : (fh + 1) * 384],
                                start=(h == 0),
                                stop=(h == H - 1),
                            )
                        outs = misc.tile(
                            [P, 384], F32, tag="outs", bufs=2, name=f"ou{g}{fh}"
                        )
                        nc.vector.tensor_add(
                            out=outs, in0=op, in1=bob_sb[:, fh * 384 : (fh + 1) * 384]
                        )
                        nc.sync.dma_start(
                            out=y[g * P : (g + 1) * P, fh * 384 : (fh + 1) * 384],
                            in_=outs,
                        )

                    return emit

                return [(0, part_recip), (4, part_norm), (6, out_proj(0)),
                        (8, out_proj(1))]

            pending = []  # (slot, fn) for the previous chunk's epilogue
            for g in range(n_chunks_run):
                nslot = SLOTS[g]
                # single merged ctx accumulator for both head groups: [65, 1536]
                ctx_t = ctx_p.tile(
                    [DV, 2 * NPAIR * P], F32, tag="ctx", name=f"ctx_g{g}"
                )

                def ctx_fence(start):
                    # bank-wide zero matmuls fencing the per-head accumulation:
                    # one start=True / stop=True group per PSUM bank, with all
                    # real ctx matmuls as flags=0 accumulates in between. The
                    # full-bank writes give WAW deps ordering them correctly.
                    for lo in range(0, 2 * NPAIR * P, 512):
                        nc.tensor.matmul(
                            out=ctx_t[0:DV, lo : lo + 512],
                            lhsT=zb[0:1, 0:DV],
                            rhs=zb[0:1, 0:512],
                            start=start,
                            stop=not start,
                        )

                def emit_scores(s):
                    is_diag = s == nslot - 1
                    pj, pcol = s >> 3, s & 7
                    pts = []
                    for hg in range(2):
                        sc = sc_p.tile(
                            [P, NPAIR * P], F32, tag="sc", name=f"sc_{g}_{s}_{hg}"
                        )
                        for pl in range(3):
                            pc = hg * 3 + pl
                            if is_diag:
                                lhsT = kto[:, pc, g * P : (g + 1) * P]
                            else:
                                lhsT = ktp[pj][:, pc, pcol * P : (pcol + 1) * P]
                            # single-shot scores; interleaved start=True groups
                            # in one bank are HW-safe. rhs [128, 2, 128] = both
                            # zero-padded Q variants -> out [128, 256].
                            nc.tensor.matmul(
                                out=sc[:, pl * 2 * P : (pl + 1) * 2 * P],
                                lhsT=lhsT,
                                rhs=qtp[:, pc, :, g * P : (g + 1) * P],
                                start=True,
                                stop=True,
                                skip_group_check=True,
                            )
                        pt = pt_p.tile(
                            [P, NPAIR * P], BF16, tag="pt", bufs=3,
                            name=f"pt_{g}_{s}_{hg}",
                        )
                        sg_idx = SLOT_BASE[g] + s
                        nc.scalar.activation(
                            out=pt,
                            in_=sc,
                            func=mybir.ActivationFunctionType.Exp,
                            bias=btab_sb[:, sg_idx : sg_idx + 1],
                            scale=0.125,
                        )
                        if is_diag:
                            ptm = pt_p.tile(
                                [P, NPAIR * P], BF16, tag="ptm", name=f"ptm_{g}_{hg}"
                            )
                            nc.vector.tensor_mul(out=ptm, in0=pt, in1=dmask_bf)
                            pt = ptm
                        pts.append(pt)
                    return pts

                def emit_attnv(s, pts):
                    is_diag = s == nslot - 1
                    pj, pcol = s >> 3, s & 7
                    for hg in range(2):
                        for hl in range(6):
                            h = hg * 6 + hl
                            vsrc = (
                                vto_v[:, g, h, :]
                                if is_diag
                                else vtp_v[pj][:, pcol, h, :]
                            )
                            nc.tensor.matmul(
                                out=ctx_t[:, (hg * 6 + hl) * P : (hg * 6 + hl + 1) * P],
                                lhsT=vsrc,
                                rhs=pts[hg][:, hl * P : (hl + 1) * P],
                                start=False,
                                stop=False,
                            )

                # slots 0/1's scores+exp go ahead of the fence: the fence's
                # WAW on the ctx buffer waits for the previous chunk's SBUF
                # evacuation, and ACT must stay fed across that hop.
                pts0 = emit_scores(0)
                pts1 = emit_scores(1)
                ctx_fence(start=True)
                emit_attnv(0, pts0)
                for slot_at, fn in pending:
                    if slot_at <= 1:
                        fn()
                pending = [(sl, fn) for sl, fn in pending if sl > 1]
                emit_attnv(1, pts1)
                for s in range(2, nslot):
                    if g == 0 and s == 4:
                        # piece 0 second half MUST precede slot 4's scores in
                        # the in-order PE stream (they read its output); the
                        # exp backlog covers part of the projection burst.
                        project_piece0(1, xt0[:, 1, :, :])
                    pts = emit_scores(s)
                    for slot_at, fn in pending:
                        if slot_at == s:
                            fn()
                    pending = [(sl, fn) for sl, fn in pending if sl != s]
                    emit_attnv(s, pts)
                ctx_fence(start=False)
                # evacuate ctx to SBUF immediately: releases the PSUM
                # accumulator so the next chunk's fence isn't blocked.
                ctxs = misc.tile(
                    [DV, 2 * NPAIR * P], F32, tag="ctxs", bufs=1, name=f"cs{g}"
                )
                nc.vector.tensor_copy(out=ctxs, in_=ctx_t)
                pending = make_epilogue(g, ctxs, last=(g == n_chunks_run - 1))
            for _, fn in pending:
                fn()

    nc.compile()
    return nc


_NC_CACHE = None


def _get_program():
    global _NC_CACHE
    if _NC_CACHE is None:
        _NC_CACHE = build_program()
    return _NC_CACHE


def _host_inputs(x, Wq, Wk, Wv, Wo, bo):
    """Build per-core input maps (host does dtype casts + transposes only)."""
    BF = ml_dtypes.bfloat16
    x = np.ascontiguousarray(x.reshape(S, E), dtype=np.float32)
    # x^T piece 0 in [P, 2, EC, 512] layout: [p, sg, c, j] = x[sg*512+j, c*128+p]
    xt0 = np.ascontiguousarray(
        x[0:1024].reshape(2, 512, EC, P).transpose(3, 0, 2, 1).astype(BF)
    )

    def w_arrange(W):
        return np.ascontiguousarray(
            np.asarray(W, dtype=np.float32).reshape(EC, P, E).transpose(1, 0, 2).astype(BF)
        )

    wq_a, wk_a, wv_a = w_arrange(Wq), w_arrange(Wk), w_arrange(Wv)
    wo_a = np.ascontiguousarray(
        np.asarray(Wo, dtype=np.float32).reshape(H, D, E).transpose(1, 0, 2).astype(BF)
    )
    bob = np.ascontiguousarray(np.broadcast_to(bo.astype(np.float32), (P, E)))
    tri = (np.arange(P)[:, None] <= np.arange(P)[None, :]).astype(np.float32)
    dmask = np.ascontiguousarray(np.tile(tri, (1, NPAIR)).astype(BF))

    xT = x.T.reshape(EC, P, S)  # [c, p, s]

    in_maps = []
    for c in range(N_CORES):
        chunks = [8 * g + c for g in range(NCH)]
        xqt = np.empty((P, EC, QOWN), dtype=BF)
        for g, gc in enumerate(chunks):
            xqt[:, :, g * P : (g + 1) * P] = (
                xT[:, :, gc * P : (gc + 1) * P].transpose(1, 0, 2).astype(BF)
            )
        btab = np.zeros((P, TOT_SLOTS), dtype=np.float32)
        for g in range(NCH):
            diagk = chunks[g]
            for s in range(SLOTS[g]):
                if s == SLOTS[g] - 1 or s < diagk:
                    v = 0.0  # diagonal slot or fully-valid block
                else:
                    v = NEG  # causally dead block
                btab[:, SLOT_BASE[g] + s] = v
        in_maps.append(
            {
                "xt0": xt0,
                "xqt": np.ascontiguousarray(xqt),
                "wq": wq_a,
                "wk": wk_a,
                "wv": wv_a,
                "wo": wo_a,
                "bob": bob,
                "btab": btab,
                "dmask": dmask,
            }
        )
    return in_maps


def kernel(x, Wq, Wk, Wv, Wo, bo, mask=None, **_ignored):
    nc = _get_program()
    in_maps = _host_inputs(
        np.asarray(x), np.asarray(Wq), np.asarray(Wk), np.asarray(Wv),
        np.asarray(Wo), np.asarray(bo),
    )
    trace = bool(int(os.environ.get("BASS_KERNEL_TRACE", "0")))
    res = bass_utils.run_bass_kernel_spmd(
        nc, in_maps, core_ids=list(range(N_CORES)), trace=trace
    )
    if trace:
        kernel.last_results = res
    out = np.empty((S, E), dtype=np.float32)
    for c in range(N_CORES):
        yc = res.results[c]["y"]
        for g in range(NCH):
            gc = 8 * g + c
            out[gc * P : (gc + 1) * P] = yc[g * P : (g + 1) * P]
    return out.reshape(1, S, E)


# revision 14
# speedup vs baseline: 2.7485x; 1.0644x over previous
"""Trainium2 Bass kernel for nn_MultiHeadAttention (B=1, S=4096, E=768, H=12, D=64).

Causal multi-head attention, sequence-parallel across 8 NeuronCores.

Strategy (single SPMD program, per-core variation is data-only):
- Query rows are split into 32 global chunks of 128 rows. Core c owns chunks
  G(g) = 8g + c for g = 0..3. Chunk g runs a fixed slot loop of 8(g+1)
  k-blocks (uniform across cores); causally-dead slots are killed by a
  per-slot exp bias of -60 (data), and the diagonal block is handled in the
  last slot with separately-projected "own" K/V tiles plus a triangular mask.
- K/V are sequence-sharded: each core projects K^T/V only for its own 512
  q-rows (which double as its K/V shard). Pieces 1-3 (rows 1024:4096) are
  exchanged via three pipelined 8-core AllGathers (HBM bounce buffers,
  TOPSP/SDMA move the data while the engines compute); piece 0 (rows 0:1024)
  is cheaper to recompute locally than to wait for a gather, and its second
  half is interleaved after chunk 0's first slots so the exp stream starts
  as early as possible.
- Scores are computed transposed (S^T[k, q], k on partitions) so the exp
  output P^T feeds the attn@V matmul directly. Row sums l come from a fused
  ones-column appended to V. The epilogue is ACT-free (DVE
  reciprocal_approx_fast + K=1 broadcast matmul + DVE multiply): the scalar
  engine runs nothing but the exp stream - one table set loaded once - and
  exp is the slot-loop bottleneck at 2 x (768+352)/1.2 ns per slot.
- Epilogues are deferred and split: the DVE/DMA-only reciprocal chain is
  emitted at the next chunk's start, and the PE pieces (broadcast, out-proj
  halves) at slots 4/6/8 of the next chunk, when their inputs are guaranteed
  resolved - so the in-order PE queue never parks on a not-yet-ready
  epilogue matmul and the HAM clock gate stays open.
"""

import os
from contextlib import ExitStack

import numpy as np
import ml_dtypes

import concourse.bass as bass
import concourse.tile as tile
from concourse import bacc, bass_utils, mybir

F32 = mybir.dt.float32
BF16 = mybir.dt.bfloat16

N_CORES = 8
S, E, H, D = 4096, 768, 12, 64
P = 128
NCH = 4  # chunks per core
SLOTS = [8, 16, 24, 32]  # slot count for chunk g
SLOT_BASE = [0, 8, 24, 48]  # cumulative
TOT_SLOTS = 80
EC = E // P  # 6 e-chunks of 128
NPAIR = 6  # head pairs
QOWN = NCH * P  # 512 own q rows
NEG = -60.0  # exp bias for masked slots: exp(-60 +- 4) == 0 numerically
DV = D + 1  # v row block: 64 dims + ones col
AGW = NPAIR * P + H * DV  # 768 + 780 = 1548 allgather row width


def build_program():
    nc = bacc.Bacc("TRN2", target_bir_lowering=False, debug=False, num_devices=N_CORES)

    # host-prepped inputs (bf16, pre-transposed / pre-arranged)
    xt0 = nc.dram_tensor("xt0", [P, 2, EC, 512], BF16, kind="ExternalInput").ap()
    xqt = nc.dram_tensor("xqt", [P, EC, QOWN], BF16, kind="ExternalInput").ap()
    wq = nc.dram_tensor("wq", [P, EC, E], BF16, kind="ExternalInput").ap()
    wk = nc.dram_tensor("wk", [P, EC, E], BF16, kind="ExternalInput").ap()
    wv = nc.dram_tensor("wv", [P, EC, E], BF16, kind="ExternalInput").ap()
    wo = nc.dram_tensor("wo", [D, H, E], BF16, kind="ExternalInput").ap()
    bob = nc.dram_tensor("bob", [P, E], F32, kind="ExternalInput").ap()
    btab = nc.dram_tensor("btab", [P, TOT_SLOTS], F32, kind="ExternalInput").ap()
    dmask = nc.dram_tensor("dmask", [P, NPAIR * P], BF16, kind="ExternalInput").ap()
    y = nc.dram_tensor("y", [QOWN, E], F32, kind="ExternalOutput").ap()

    with tile.TileContext(nc) as tc, ExitStack() as top:
        const = top.enter_context(tc.tile_pool(name="const", bufs=1))
        big = top.enter_context(tc.tile_pool(name="big", bufs=1))
        own = top.enter_context(tc.tile_pool(name="own", bufs=1))
        kvw = top.enter_context(tc.tile_pool(name="kvw", bufs=1))
        xt_p = top.enter_context(tc.tile_pool(name="xtp", bufs=2))
        dram = top.enter_context(tc.tile_pool(name="dram", bufs=1, space="DRAM"))
        sc_p = top.enter_context(tc.tile_pool(name="scp", bufs=2, space="PSUM"))
        ctx_p = top.enter_context(tc.tile_pool(name="ctxp", bufs=1, space="PSUM"))
        epi_p = top.enter_context(tc.tile_pool(name="epip", bufs=1, space="PSUM"))

        # K/V weights live for the whole kernel (piece-0 second half is
        # projected inside the attention loop); wk + xqt are DMA'd FIRST so
        # the own-shard K projection starts as early as possible.
        wkb = kvw.tile([P, EC, E], BF16, tag="wkb")
        nc.sync.dma_start(out=wkb, in_=wk)
        wvb = kvw.tile([P, EC, E], BF16, tag="wvb")
        nc.sync.dma_start(out=wvb, in_=wv)

        # ---- persistent operands ----
        ktp = [
            big.tile([P, NPAIR, 1024], BF16, tag=f"kt{j}", name=f"kt{j}")
            for j in range(4)
        ]
        vtp = [
            big.tile([P, 8, H * DV], BF16, tag=f"vt{j}", name=f"vt{j}")
            for j in range(4)
        ]
        vtp_v = [t.rearrange("p b (h c) -> p b h c", c=DV) for t in vtp]
        qtp = own.tile([P, NPAIR, 2, QOWN], BF16, tag="qtp")
        nc.vector.memset(qtp[D:P, :, 0, :], 0.0)
        nc.vector.memset(qtp[0:D, :, 1, :], 0.0)
        kto = own.tile([P, NPAIR, QOWN], BF16, tag="kto")  # own K^T (diag + AG src)
        vto = own.tile([P, NCH, H * DV], BF16, tag="vto")  # own ones+V
        vto_v = vto.rearrange("p b (h c) -> p b h c", c=DV)
        nc.vector.memset(vto_v[:, :, :, D : D + 1], 1.0)
        for j in range(4):
            nc.vector.memset(vtp_v[j][:, :, :, D : D + 1], 1.0)

        ones_sb = const.tile([P, D], F32, tag="ones")
        nc.vector.memset(ones_sb, 1.0)
        zb = const.tile([P, 512], BF16, tag="zb")
        nc.vector.memset(zb, 0.0)

        def project_pairs(w_bf, dst, src_t, ncols, split=False):
            for pc in range(NPAIR):
                ps = sc_p.tile([P, ncols], F32, tag="sc", name=f"ps_p{pc}")
                for ec in range(EC):
                    nc.tensor.matmul(
                        out=ps,
                        lhsT=w_bf[:, ec, pc * P : (pc + 1) * P],
                        rhs=src_t[:, ec, :],
                        start=(ec == 0),
                        stop=(ec == EC - 1),
                    )
                if split:
                    nc.vector.tensor_copy(out=dst[0:D, pc, 0, :], in_=ps[0:D, :])
                    nc.vector.tensor_copy(out=dst[D:P, pc, 1, :], in_=ps[D:P, :])
                else:
                    nc.scalar.copy(out=dst[:, pc, :], in_=ps)

        def project_v(dst_view, xt_tile, xt_col0):
            for half in range(2):
                ps = sc_p.tile([P, E // 2], F32, tag="sc", name=f"ps_v{half}")
                for ec in range(EC):
                    nc.tensor.matmul(
                        out=ps,
                        lhsT=xt_tile[:, ec, xt_col0 : xt_col0 + P],
                        rhs=wvb[:, ec, half * 384 : (half + 1) * 384],
                        start=(ec == 0),
                        stop=(ec == EC - 1),
                    )
                nc.vector.tensor_copy(
                    out=dst_view[:, half * 6 : (half + 1) * 6, 0:D],
                    in_=ps.rearrange("p (h d) -> p h d", d=D),
                )

        xt_g = [None, None]

        def fetch_piece0(sg):
            xt_g[sg] = xt_p.tile([P, EC, 512], BF16, tag="xtg", name=f"xt_{sg}")
            nc.sync.dma_start(out=xt_g[sg], in_=xt0[:, sg, :, :])

        def project_p0_k(sg, lo, n):
            # K^T for piece-0 columns [sg*512+lo, +n)
            for pc in range(NPAIR):
                ps = sc_p.tile([P, n], F32, tag="sc", name=f"p0k{sg}{pc}{lo}")
                for ec in range(EC):
                    nc.tensor.matmul(
                        out=ps,
                        lhsT=wkb[:, ec, pc * P : (pc + 1) * P],
                        rhs=xt_g[sg][:, ec, lo : lo + n],
                        start=(ec == 0),
                        stop=(ec == EC - 1),
                    )
                nc.scalar.copy(
                    out=ktp[0][:, pc, sg * 512 + lo : sg * 512 + lo + n], in_=ps
                )

        def project_p0_v(sg, b0, nb):
            for b in range(b0, b0 + nb):
                project_v(vtp_v[0][:, 4 * sg + b, :, :], xt_g[sg], b * P)

        # ================= projection phase =================
        with ExitStack() as proj:
            wq_pool = proj.enter_context(tc.tile_pool(name="wqp", bufs=1))
            xq_pool = proj.enter_context(tc.tile_pool(name="xqp", bufs=1))

            xqt_sb = xq_pool.tile([P, EC, QOWN], BF16, tag="xqt")
            nc.sync.dma_start(out=xqt_sb, in_=xqt)
            wqb = wq_pool.tile([P, EC, E], BF16, tag="wqb")
            nc.sync.dma_start(out=wqb, in_=wq)

            # own-shard K/V first: they feed the AllGathers
            project_pairs(wkb, kto, xqt_sb, QOWN)
            for qb in range(NCH):
                project_v(vto_v[:, qb, :, :], xqt_sb, qb * P)

            # bounce own chunks 1-3 out; trigger AG j then immediately queue
            # its scatter DMAs (they wait on the collective's completion).
            ag_in = [
                dram.tile([P, AGW], BF16, tag=f"agi{j}", name=f"agi{j}")
                for j in range(1, 4)
            ]
            ag_out = [
                dram.tile(
                    [N_CORES * P, AGW], BF16, tag=f"ago{j}", name=f"ago{j}",
                    addr_space="Shared",
                )
                for j in range(1, 4)
            ]
            for j in range(1, 4):
                bi = ag_in[j - 1]
                nc.gpsimd.dma_start(
                    out=bi[:, 0 : NPAIR * P].rearrange("p (c q) -> p c q", q=P),
                    in_=kto[:, :, j * P : (j + 1) * P],
                )
                nc.gpsimd.dma_start(out=bi[:, NPAIR * P : AGW], in_=vto[:, j, :])
                nc.gpsimd.collective_compute(
                    "AllGather",
                    mybir.AluOpType.bypass,
                    replica_groups=[list(range(N_CORES))],
                    ins=[bi[:].opt()],
                    outs=[ag_out[j - 1][:].opt()],
                )
            for j in range(1, 4):
                ao = ag_out[j - 1]
                for r in range(N_CORES):
                    nc.gpsimd.dma_start(
                        out=ktp[j][:, :, r * P : (r + 1) * P],
                        in_=ao[r * P : (r + 1) * P, 0 : NPAIR * P].rearrange(
                            "p (c q) -> p c q", q=P
                        ),
                    )
                    nc.gpsimd.dma_start(
                        out=vtp[j][:, r, :],
                        in_=ao[r * P : (r + 1) * P, NPAIR * P : AGW],
                    )

            # own Q while the gathers fly
            project_pairs(wqb, qtp, xqt_sb, QOWN, split=True)

            # full piece 0 (rows 0:1024): cheaper before the slot loop than
            # as inserts - the ACT exp backlog is capped at ~2 slots, so any
            # mid-stream PE insert stalls the exp stream by nearly its full
            # serial time.
            fetch_piece0(0)
            fetch_piece0(1)
            project_p0_k(0, 0, 512)
            project_p0_v(0, 0, 4)
            project_p0_k(1, 0, 512)
            project_p0_v(1, 0, 4)

        phase = os.environ.get("KERNEL_PHASE", "full")
        n_chunks_run = 0 if phase == "proj" else NCH
        if phase.startswith("att"):
            n_chunks_run = int(phase[3:])

        # ================= attention phase =================
        with ExitStack() as att:
            wo_pool = att.enter_context(tc.tile_pool(name="wop", bufs=1))
            wob = wo_pool.tile([D, H, E], BF16, tag="wob")
            nc.sync.dma_start(out=wob, in_=wo)
            btab_sb = const.tile([P, TOT_SLOTS], F32, tag="btab")
            nc.sync.dma_start(out=btab_sb, in_=btab)
            dmask_bf = const.tile([P, NPAIR * P], BF16, tag="dmaskb")
            nc.sync.dma_start(out=dmask_bf, in_=dmask)
            bob_sb = const.tile([P, E], F32, tag="bob")
            nc.sync.dma_start(out=bob_sb, in_=bob)

            pt_p = att.enter_context(tc.tile_pool(name="pt", bufs=1))
            misc = att.enter_context(tc.tile_pool(name="misc", bufs=1))

            def make_epilogue(g, ctxs, last=False):
                """Deferred epilogue for chunk g, as (slot, fn) emission parts.
                ctxs is the SBUF ctx copy [DV, 2*NPAIR*P]; row 64 is l."""
                lr = misc.tile([1, 2 * NPAIR * P], F32, tag="lr", bufs=1, name=f"lr{g}")
                linv = misc.tile(
                    [1, 2 * NPAIR * P], F32, tag="linv", bufs=1, name=f"li{g}"
                )
                ctxn = []

                def part_recip():
                    # DVE/DMA only - no PE instructions, so the PE queue never
                    # parks on this chain.
                    nc.sync.dma_start(out=lr[0:1, :], in_=ctxs[D:DV, :])
                    nc.vector.reciprocal_approx_fast(out=linv[0:1, :], in_=lr[0:1, :])

                def part_norm():
                    for hg in range(2):
                        cn = misc.tile(
                            [D, NPAIR * P], BF16, tag=f"ctxn{hg}", bufs=1,
                            name=f"cn{g}{hg}",
                        )
                        for lo, n in ((0, 512), (512, 256)):  # 1-bank pieces
                            bc = epi_p.tile(
                                [D, n], F32, tag="epi", name=f"bc{g}{hg}{lo}"
                            )
                            nc.tensor.matmul(
                                out=bc,
                                lhsT=ones_sb[0:1, 0:D],
                                rhs=linv[0:1, hg * 768 + lo : hg * 768 + lo + n],
                                start=True,
                                stop=True,
                            )
                            nc.vector.tensor_mul(
                                out=cn[:, lo : lo + n],
                                in0=ctxs[0:D, hg * 768 + lo : hg * 768 + lo + n],
                                in1=bc,
                            )
                        ctxn.append(cn)

                op_tile = [None]

                def op_pair(i):
                    # two out-proj matmuls per slot: each dose stays under the
                    # per-slot PE slack so the exp stream never stalls.
                    fh, hp = i // 6, i % 6

                    def emit():
                        if hp == 0:
                            op_tile[0] = (
                                sc_p.tile([P, 384], F32, tag="sc", name=f"op{g}{fh}")
                                if last
                                else epi_p.tile(
                                    [P, 384], F32, tag="epi", name=f"op{g}{fh}"
                                )
                            )
                        op = op_tile[0]
                        for h in (2 * hp, 2 * hp + 1):
                            nc.tensor.matmul(
                                out=op,
                                lhsT=ctxn[h // 6][:, (h % 6) * P : (h % 6 + 1) * P],
                                rhs=wob[:, h, fh * 384 : (fh + 1) * 384],
                                start=(h == 0),
                                stop=(h == H - 1),
                                skip_group_check=True,
                            )
                        if hp == 5:
                            outs = misc.tile(
                                [P, 384], F32, tag="outs", bufs=2, name=f"ou{g}{fh}"
                            )
                            nc.vector.tensor_add(
                                out=outs,
                                in0=op,
                                in1=bob_sb[:, fh * 384 : (fh + 1) * 384],
                            )
                            nc.sync.dma_start(
                                out=y[g * P : (g + 1) * P, fh * 384 : (fh + 1) * 384],
                                in_=outs,
                            )

                    return emit

                return (
                    [(0, part_recip), (3, part_norm)]
                    + [(4 + i, op_pair(i)) for i in range(12)]
                )

            pending = []  # (slot, fn) for the previous chunk's epilogue
            for g in range(n_chunks_run):
                nslot = SLOTS[g]
                # single merged ctx accumulator for both head groups: [65, 1536]
                ctx_t = ctx_p.tile(
                    [DV, 2 * NPAIR * P], F32, tag="ctx", name=f"ctx_g{g}"
                )

                def ctx_fence(start):
                    # bank-wide zero matmuls fencing the per-head accumulation:
                    # one start=True / stop=True group per PSUM bank, with all
                    # real ctx matmuls as flags=0 accumulates in between. The
                    # full-bank writes give WAW deps ordering them correctly.
                    for lo in range(0, 2 * NPAIR * P, 512):
                        nc.tensor.matmul(
                            out=ctx_t[0:DV, lo : lo + 512],
                            lhsT=zb[0:1, 0:DV],
                            rhs=zb[0:1, 0:512],
                            start=start,
                            stop=not start,
                        )

                def emit_scores(s):
                    is_diag = s == nslot - 1
                    pj, pcol = s >> 3, s & 7
                    pts = []
                    for hg in range(2):
                        sc = sc_p.tile(
                            [P, NPAIR * P], F32, tag="sc", name=f"sc_{g}_{s}_{hg}"
                        )
                        for pl in range(3):
                            pc = hg * 3 + pl
                            if is_diag:
                                lhsT = kto[:, pc, g * P : (g + 1) * P]
                            else:
                                lhsT = ktp[pj][:, pc, pcol * P : (pcol + 1) * P]
                            # single-shot scores; interleaved start=True groups
                            # in one bank are HW-safe. rhs [128, 2, 128] = both
                            # zero-padded Q variants -> out [128, 256].
                            nc.tensor.matmul(
                                out=sc[:, pl * 2 * P : (pl + 1) * 2 * P],
                                lhsT=lhsT,
                                rhs=qtp[:, pc, :, g * P : (g + 1) * P],
                                start=True,
                                stop=True,
                                skip_group_check=True,
                            )
                        pt = pt_p.tile(
                            [P, NPAIR * P], BF16, tag="pt", bufs=4,
                            name=f"pt_{g}_{s}_{hg}",
                        )
                        sg_idx = SLOT_BASE[g] + s
                        nc.scalar.activation(
                            out=pt,
                            in_=sc,
                            func=mybir.ActivationFunctionType.Exp,
                            bias=btab_sb[:, sg_idx : sg_idx + 1],
                            scale=0.125,
                        )
                        if is_diag:
                            ptm = pt_p.tile(
                                [P, NPAIR * P], BF16, tag="ptm", name=f"ptm_{g}_{hg}"
                            )
                            nc.vector.tensor_mul(out=ptm, in0=pt, in1=dmask_bf)
                            pt = ptm
                        pts.append(pt)
                    return pts

                def emit_attnv(s, pts):
                    is_diag = s == nslot - 1
                    pj, pcol = s >> 3, s & 7
                    for hg in range(2):
                        for hl in range(6):
                            h = hg * 6 + hl
                            vsrc = (
                                vto_v[:, g, h, :]
                                if is_diag
                                else vtp_v[pj][:, pcol, h, :]
                            )
                            nc.tensor.matmul(
                                out=ctx_t[:, (hg * 6 + hl) * P : (hg * 6 + hl + 1) * P],
                                lhsT=vsrc,
                                rhs=pts[hg][:, hl * P : (hl + 1) * P],
                                start=False,
                                stop=False,
                            )

                # slots 0/1's scores+exp go ahead of the fence: the fence's
                # WAW on the ctx buffer waits for the previous chunk's SBUF
                # evacuation, and ACT must stay fed across that hop.
                pts0 = emit_scores(0)
                pts1 = emit_scores(1)
                ctx_fence(start=True)
                emit_attnv(0, pts0)
                for slot_at, fn in pending:
                    if slot_at <= 1:
                        fn()
                pending = [(sl, fn) for sl, fn in pending if sl > 1]
                emit_attnv(1, pts1)
                for s in range(2, nslot):
                    pts = emit_scores(s)
                    for slot_at, fn in pending:
                        if slot_at == s:
                            fn()
                    pending = [(sl, fn) for sl, fn in pending if sl != s]
                    emit_attnv(s, pts)
                ctx_fence(start=False)
                # evacuate ctx to SBUF immediately: releases the PSUM
                # accumulator so the next chunk's fence isn't blocked.
                ctxs = misc.tile(
                    [DV, 2 * NPAIR * P], F32, tag="ctxs", bufs=1, name=f"cs{g}"
                )
                nc.vector.tensor_copy(out=ctxs, in_=ctx_t)
                pending = make_epilogue(g, ctxs, last=(g == n_chunks_run - 1))
            for _, fn in pending:
                fn()

    nc.compile()
    return nc


_NC_CACHE = None


def _get_program():
    global _NC_CACHE
    if _NC_CACHE is None:
        _NC_CACHE = build_program()
    return _NC_CACHE


def _host_inputs(x, Wq, Wk, Wv, Wo, bo):
    """Build per-core input maps (host does dtype casts + transposes only)."""
    BF = ml_dtypes.bfloat16
    x = np.ascontiguousarray(x.reshape(S, E), dtype=np.float32)
    # x^T piece 0 in [P, 2, EC, 512] layout: [p, sg, c, j] = x[sg*512+j, c*128+p]
    xt0 = np.ascontiguousarray(
        x[0:1024].reshape(2, 512, EC, P).transpose(3, 0, 2, 1).astype(BF)
    )

    def w_arrange(W):
        return np.ascontiguousarray(
            np.asarray(W, dtype=np.float32).reshape(EC, P, E).transpose(1, 0, 2).astype(BF)
        )

    wq_a, wk_a, wv_a = w_arrange(Wq), w_arrange(Wk), w_arrange(Wv)
    wo_a = np.ascontiguousarray(
        np.asarray(Wo, dtype=np.float32).reshape(H, D, E).transpose(1, 0, 2).astype(BF)
    )
    bob = np.ascontiguousarray(np.broadcast_to(bo.astype(np.float32), (P, E)))
    tri = (np.arange(P)[:, None] <= np.arange(P)[None, :]).astype(np.float32)
    dmask = np.ascontiguousarray(np.tile(tri, (1, NPAIR)).astype(BF))

    xT = x.T.reshape(EC, P, S)  # [c, p, s]

    in_maps = []
    for c in range(N_CORES):
        chunks = [8 * g + c for g in range(NCH)]
        xqt = np.empty((P, EC, QOWN), dtype=BF)
        for g, gc in enumerate(chunks):
            xqt[:, :, g * P : (g + 1) * P] = (
                xT[:, :, gc * P : (gc + 1) * P].transpose(1, 0, 2).astype(BF)
            )
        btab = np.zeros((P, TOT_SLOTS), dtype=np.float32)
        for g in range(NCH):
            diagk = chunks[g]
            for s in range(SLOTS[g]):
                if s == SLOTS[g] - 1 or s < diagk:
                    v = 0.0  # diagonal slot or fully-valid block
                else:
                    v = NEG  # causally dead block
                btab[:, SLOT_BASE[g] + s] = v
        in_maps.append(
            {
                "xt0": xt0,
                "xqt": np.ascontiguousarray(xqt),
                "wq": wq_a,
                "wk": wk_a,
                "wv": wv_a,
                "wo": wo_a,
                "bob": bob,
                "btab": btab,
                "dmask": dmask,
            }
        )
    return in_maps


def kernel(x, Wq, Wk, Wv, Wo, bo, mask=None, **_ignored):
    nc = _get_program()
    in_maps = _host_inputs(
        np.asarray(x), np.asarray(Wq), np.asarray(Wk), np.asarray(Wv),
        np.asarray(Wo), np.asarray(bo),
    )
    trace = bool(int(os.environ.get("BASS_KERNEL_TRACE", "0")))
    res = bass_utils.run_bass_kernel_spmd(
        nc, in_maps, core_ids=list(range(N_CORES)), trace=trace
    )
    if trace:
        kernel.last_results = res
    out = np.empty((S, E), dtype=np.float32)
    for c in range(N_CORES):
        yc = res.results[c]["y"]
        for g in range(NCH):
            gc = 8 * g + c
            out[gc * P : (gc + 1) * P] = yc[g * P : (g + 1) * P]
    return out.reshape(1, S, E)
